# revision 37
# baseline (speedup 1.0000x reference)
"""Trainium2 Bass kernel for the CompILE-style model (nn_CompILE_5111011082477).

Sharding: pure data-parallel over batch B=32 across 8 cores (B=4 per core),
all parameters replicated, zero collectives.

Device program (per core) highlights:
  - All feed-forward matmuls run "activation transposed": features on
    partitions, tokens on the free dim, weights stationary.
  - Host folds the embedding into effective weights:
      Gin = x @ (embed_W @ W_ih.T[:256]) + onehot(a) @ (table @ W_ih.T[256:] + bias)
    (one-hot rows sum to 1, so all biases fold into the action table term).
  - LSTM recurrence: per step, gates land in PSUM as [(j,b) partitions,
    (gate, hh) free] via 4-way column-tiled matmuls (tile_position=(0,32j)).
    Gin is injected as the start=True matmul using an identity-selector lhsT.
    Cell math runs full-width on ACT/DVE; one PE transpose regenerates h^T.
  - Decoder (l=16 parallel MLPs) computed ONCE (it is segment-independent)
    and mixed with each segment's sample_z via small K=16 matmuls.
"""

import os
import numpy as np

B_FULL, T, D, A, H, L = 32, 128, 128, 32, 512, 16
NC = 8
B = B_FULL // NC          # 4 per core
SEGS = 4
G = 4 * H                 # 2048
HH = 128                  # h per j-slice
J = 4                     # h slices
KC = 4                    # contraction chunks of H
TOK = B * T               # 512 per core
NEG_INF = -1e30
EPS = 1e-17

_prog_cache = {}


def last_exec_time_ns():
    return _prog_cache.get("exec_time_ns")


# ----------------------------------------------------------------------------
# Host-side packing
# ----------------------------------------------------------------------------

def _gumbel_noise():
    """Bit-identical gumbel noise to reference (key 42, CPU)."""
    import jax
    cpu = jax.devices("cpu")[0]
    with jax.default_device(cpu):
        nkey = jax.random.key(42)
        gb = np.stack([
            np.asarray(jax.random.gumbel(jax.random.fold_in(nkey, 2 * s),
                                         (B_FULL, T), jax.numpy.float32))
            for s in range(SEGS - 1)])            # [3, 32, 128]
        gz = np.stack([
            np.asarray(jax.random.gumbel(jax.random.fold_in(nkey, 2 * s + 1),
                                         (B_FULL, L), jax.numpy.float32))
            for s in range(SEGS)])                # [4, 32, 16]
    return gb, gz


def _pack_weights(inp):
    """Build all shared (replicated) device tensors. float64 intermediates."""
    f = np.float32
    embed_W = inp["embed_W"].astype(np.float64)        # [128, 256]
    embed_b = inp["embed_b"].astype(np.float64)        # [256]
    table = inp["embed_action_table"].astype(np.float64)  # [32, 256]
    W_ih = inp["W_ih"].astype(np.float64)              # [2048, 512]
    W_hh = inp["W_hh"].astype(np.float64)              # [2048, 512]
    b_ih = inp["b_ih"].astype(np.float64)
    b_hh = inp["b_hh"].astype(np.float64)

    WihT = W_ih.T                                      # [512, 2048]
    M1 = embed_W @ WihT[:256]                          # [128, 2048] gate idx g*512+h'
    bias_g = b_ih + b_hh + embed_b @ WihT[:256]        # [2048]
    M2p = table @ WihT[256:] + bias_g[None, :]         # [32, 2048]

    def gate_reindex(M):  # [..., g*512 + j*128 + hh] -> [..., (j*4+g)*128 + hh]
        Mr = M.reshape(M.shape[0], 4, J, HH)           # [in, g, j, hh]
        return np.ascontiguousarray(Mr.transpose(0, 2, 1, 3).reshape(M.shape[0], G))

    M1_dev = gate_reindex(M1).astype(f)
    M2p_dev = gate_reindex(M2p).astype(f)

    # Whh_dev[kk, ((k*4+j)*4+g)*128+hh] = W_hh[g*512+j*128+hh, k*128+kk]
    Whh = W_hh.reshape(4, J, HH, KC, 128)              # [g, j, hh, k, kk]
    Whh_dev = np.ascontiguousarray(
        Whh.transpose(4, 3, 1, 0, 2).reshape(128, KC * J * 4 * HH)).astype(f)

    def chunk2(Wmat):  # [512, 512] -> [128, (k*4+m)*128+mm]
        Wr = Wmat.reshape(KC, 128, 4, 128)             # [k, kk, m, mm]
        return np.ascontiguousarray(Wr.transpose(1, 0, 2, 3).reshape(128, 2048))

    hb1_dev = chunk2(inp["hb1_W"].astype(np.float64)).astype(f)
    hz1_dev = chunk2(inp["hz1_W"].astype(np.float64)).astype(f)
    hb1b_dev = np.ascontiguousarray(
        inp["hb1_b"].astype(np.float64).reshape(4, 128).T).astype(f)   # [mm, m]
    hz1b_dev = np.ascontiguousarray(
        inp["hz1_b"].astype(np.float64).reshape(4, 128).T).astype(f)
    hb2_dev = np.ascontiguousarray(
        inp["hb2_W"].astype(np.float64).reshape(KC, 128).T).astype(f)  # [kk, k]
    hb2_b = float(inp["hb2_b"][0])
    hz2 = inp["hz2_W"].astype(np.float64).reshape(KC, 128, L)          # [k, kk, l]
    hz2_dev = np.ascontiguousarray(hz2.transpose(1, 0, 2).reshape(128, KC * L)).astype(f)
    hz2b_row = inp["hz2_b"].astype(f).reshape(1, L)

    dec1 = inp["dec1_W"].astype(np.float64)            # [L, 256, 512]
    dec1b = inp["dec1_b"].astype(np.float64)           # [L, 512]
    W1e = np.einsum("dh,lhk->ldk", embed_W, dec1)      # [L, 128, 512]
    b1e = dec1b + np.einsum("h,lhk->lk", embed_b, dec1)
    # W1e_dev[dd, (l*4+m)*128+mm]
    W1e_dev = np.ascontiguousarray(
        W1e.reshape(L, 128, 4, 128).transpose(1, 0, 2, 3).reshape(128, L * 512)).astype(f)
    b1e_dev = np.ascontiguousarray(
        b1e.reshape(L, 4, 128).transpose(2, 0, 1).reshape(128, L * 4)).astype(f)

    dec2 = inp["dec2_W"].astype(np.float64)            # [L, 512, 512]
    # W2_dev[kk, ((l*4+k)*4+m)*128+mm]
    W2_dev = np.ascontiguousarray(
        dec2.reshape(L, KC, 128, 4, 128).transpose(2, 0, 1, 3, 4)
        .reshape(128, L * 2048)).astype(f)
    b2_dev = np.ascontiguousarray(
        inp["dec2_b"].astype(np.float64).reshape(L, 4, 128)
        .transpose(2, 0, 1).reshape(128, L * 4)).astype(f)

    dec3 = inp["dec3_W"].astype(np.float64)            # [L, 512, 32]
    W3_dev = np.ascontiguousarray(
        dec3.reshape(L, KC, 128, A).transpose(2, 0, 1, 3).reshape(128, L * KC * A)).astype(f)
    b3_dev = np.ascontiguousarray(
        inp["dec3_b"].astype(np.float64).T).astype(f)  # [a, l]

    I128 = np.eye(128, dtype=f)
    ones1 = np.ones((1, 4), dtype=f)

    return dict(M1=M1_dev, M2p=M2p_dev, Whh=Whh_dev, hb1=hb1_dev, hb1b=hb1b_dev,
                hb2=hb2_dev, hz1=hz1_dev, hz1b=hz1b_dev, hz2=hz2_dev,
                hz2b=hz2b_row, W1e=W1e_dev, b1e=b1e_dev, W2=W2_dev, b2=b2_dev,
                W3=W3_dev, b3=b3_dev, I128=I128, ones1=ones1), hb2_b


def _pack_core_inputs(inp, gb, gz, core):
    """Per-core activation tensors. tok col = t_hi*128 + tt*4 + b."""
    f = np.float32
    b0 = core * B
    x = np.asarray(inp["inputs"][b0:b0 + B], dtype=f)          # [4, 128, 128]
    act = np.asarray(inp["actions"][b0:b0 + B]).astype(np.int64)
    lens = np.asarray(inp["lengths"][b0:b0 + B]).astype(np.int64)

    # xT[d, t_hi*128 + tt*4 + b] = x[b, t_hi*32+tt, d]
    xr = x.reshape(B, 4, 32, D)                                # [b, t_hi, tt, d]
    xT = np.ascontiguousarray(xr.transpose(3, 1, 2, 0).reshape(D, TOK)).astype(f)
    aoh_full = np.zeros((A, B, T), dtype=f)
    for b in range(B):
        aoh_full[act[b], b, np.arange(T)] = 1.0
    aohr = aoh_full.reshape(A, B, 4, 32)                       # [a, b, t_hi, tt]
    aoh = np.ascontiguousarray(aohr.transpose(0, 2, 3, 1).reshape(A, TOK)).astype(f)

    gb_c = gb[:, b0:b0 + B, :]                                 # [3, 4, 128]
    gb_dev = np.ascontiguousarray(gb_c.reshape(1, 3 * B * T)).astype(f)
    gz_c = gz[:, b0:b0 + B, :]                                 # [4, 4, 16]
    gz_dev = np.ascontiguousarray(gz_c.transpose(1, 0, 2).reshape(B, SEGS * L)).astype(f)

    loh = np.zeros((1, B * T), dtype=f)
    for b in range(B):
        loh[0, b * T + (int(lens[b]) - 1)] = 1.0
    return dict(xT=xT, aoh=aoh, gb=gb_dev, gz=gz_dev, loh=loh)


# ----------------------------------------------------------------------------
# Device program
# ----------------------------------------------------------------------------

def _build_program(hb2_b):
    import concourse.bass as bass
    import concourse.mybir as mybir
    from concourse import bacc, tile

    f32 = mybir.dt.float32
    AF = mybir.ActivationFunctionType
    OP = mybir.AluOpType
    AX = mybir.AxisListType

    nc = bacc.Bacc(None, target_bir_lowering=False, debug=False)

    # ---- DRAM I/O ----
    di = {}
    def d_in(name, shape):
        di[name] = nc.dram_tensor(name, list(shape), f32, kind="ExternalInput")
        return di[name]

    for name, shape in [
        ("xT", (D, TOK)), ("aoh", (A, TOK)), ("M1", (D, G)), ("M2p", (A, G)),
        ("Whh", (128, KC * J * 4 * HH)), ("hb1", (128, 2048)), ("hb1b", (128, 4)),
        ("hb2", (128, 4)), ("hz1", (128, 2048)), ("hz1b", (128, 4)),
        ("hz2", (128, KC * L)), ("hz2b", (1, L)), ("W1e", (128, L * 512)),
        ("b1e", (128, L * 4)), ("W2", (128, L * 2048)), ("b2", (128, L * 4)),
        ("W3", (128, L * KC * A)), ("b3", (A, L)), ("I128", (128, 128)),
        ("ones1", (1, 4)), ("gb", (1, 3 * B * T)), ("gz", (B, SEGS * L)),
        ("loh", (1, B * T)),
    ]:
        d_in(name, shape)

    outs_dr = nc.dram_tensor("outs_dr", [L, B, T, A], f32)
    o_enc = nc.dram_tensor("o_enc", [SEGS, B, T, H], f32, kind="ExternalOutput")
    o_rec = nc.dram_tensor("o_rec", [SEGS, B, T, A], f32, kind="ExternalOutput")
    o_mask = nc.dram_tensor("o_mask", [SEGS - 1, B, T], f32, kind="ExternalOutput")
    o_lb = nc.dram_tensor("o_lb", [SEGS - 1, B, T], f32, kind="ExternalOutput")
    o_sb = nc.dram_tensor("o_sb", [SEGS, B, T], f32, kind="ExternalOutput")
    o_lz = nc.dram_tensor("o_lz", [SEGS, B, L], f32, kind="ExternalOutput")
    o_z = nc.dram_tensor("o_z", [SEGS, B, L], f32, kind="ExternalOutput")

    with tile.TileContext(nc) as tc:
        with (
            tc.tile_pool(name="w", bufs=1) as wp,
            tc.tile_pool(name="stream", bufs=2) as sp,
            tc.tile_pool(name="pg", bufs=1, space="PSUM") as pgp,
            tc.tile_pool(name="pt", bufs=1, space="PSUM") as ptp,
            tc.tile_pool(name="pd", bufs=2, space="PSUM") as pdp,
            tc.tile_pool(name="ps", bufs=2, space="PSUM") as psp,
        ):
            # ---- resident SBUF tiles + input DMAs ----
            sb = {}
            for name in ["xT", "aoh", "M1", "M2p", "Whh", "hb1", "hb1b", "hb2",
                         "hz1", "hz1b", "hz2", "hz2b", "b1e", "b2", "W3", "b3",
                         "I128", "ones1", "gb", "gz", "loh"]:
                t = wp.tile(list(di[name].shape), f32, tag=name)
                nc.sync.dma_start(t[:], di[name][:])
                sb[name] = t

            Gin = wp.tile([128, KC * J * 4 * HH], f32, tag="Gin")   # [ (tt,b), (t_hi,j,g,hh) ]
            hTs = wp.tile([128, T * 16], f32, tag="hTs")            # [hh, (t,j,b)]
            S_if = wp.tile([128, 256], f32, tag="S_if")
            S_o = wp.tile([128, 128], f32, tag="S_o")
            TC = wp.tile([128, 256], f32, tag="TC")                 # [tg | c]
            P2 = wp.tile([128, 256], f32, tag="P2")
            Cn = wp.tile([128, 128], f32, tag="Cn")
            Tc = wp.tile([128, 128], f32, tag="Tc")
            OM = wp.tile([128, 128], f32, tag="OM")
            Hb = wp.tile([128, 128], f32, tag="Hb")
            mask_sb = wp.tile([128, T], f32, tag="mask_sb")
            ones128 = wp.tile([1, 128], f32, tag="ones128")
            roT = wp.tile([128, 16], f32, tag="roT")
            hzT = wp.tile([128, 16], f32, tag="hzT")
            lb_sb = wp.tile([1, TOK], f32, tag="lb_sb")
            sbn = wp.tile([1, TOK], f32, tag="sbn")
            sbe = wp.tile([1, TOK], f32, tag="sbe")
            sb_row = wp.tile([1, TOK], f32, tag="sb_row")
            cum = wp.tile([1, TOK], f32, tag="cum")
            lncum = wp.tile([1, TOK], f32, tag="lncum")
            logacc = wp.tile([1, TOK], f32, tag="logacc")
            mask_row = wp.tile([1, TOK], f32, tag="mask_row")
            zero_row = wp.tile([1, TOK], f32, tag="zero_row")
            mx = wp.tile([1, B], f32, tag="mx")
            ssum = wp.tile([1, B], f32, tag="ssum")
            rs = wp.tile([1, B], f32, tag="rs")
            lz_sb = wp.tile([B, L], f32, tag="lz_sb")
            zn = wp.tile([B, L], f32, tag="zn")
            ze = wp.tile([B, L], f32, tag="ze")
            z_sb = wp.tile([B, L], f32, tag="z_sb")
            nm = wp.tile([B, 1], f32, tag="nm")
            zs = wp.tile([B, 1], f32, tag="zs")
            rz = wp.tile([B, 1], f32, tag="rz")
            rec_sb = wp.tile([1, 1024], f32, tag="rec_sb")
            zT_all = [wp.tile([128, B], f32, tag=f"zT{s}", name=f"zT{s}")
                      for s in range(SEGS)]

            nc.vector.memset(ones128[:], 1.0)
            eps_t = wp.tile([128, 1], f32, tag="eps_t")
            hb2b_t = wp.tile([128, 1], f32, tag="hb2b_t")
            nc.vector.memset(eps_t[:], EPS)
            nc.vector.memset(hb2b_t[:], hb2_b)
            nc.vector.memset(mask_sb[:], 1.0)
            nc.vector.memset(logacc[:], 0.0)
            nc.vector.memset(zero_row[:], 0.0)
            nc.vector.memset(Hb[:], 0.0)

            # persistent psum tiles (ping-pong)
            PG = [pgp.tile([128, 512], f32, tag=f"pg{i}", name=f"pg{i}")
                  for i in range(2)]
            PT = [ptp.tile([128, 128], f32, tag=f"pt{i}", name=f"pt{i}")
                  for i in range(2)]
            nc.vector.memset(PG[0][:], 0.0)
            nc.vector.memset(PG[1][:], 0.0)

            # ---- Gin precompute ----
            for thi in range(4):
                for j in range(J):
                    pgi = pdp.tile([128, 512], f32, tag="dec_ps")
                    nc.tensor.matmul(pgi[:], sb["xT"][:, thi * 128:(thi + 1) * 128],
                                     sb["M1"][:, j * 512:(j + 1) * 512],
                                     start=True, stop=False)
                    nc.tensor.matmul(pgi[:], sb["aoh"][:, thi * 128:(thi + 1) * 128],
                                     sb["M2p"][:, j * 512:(j + 1) * 512],
                                     start=False, stop=True)
                    nc.scalar.copy(Gin[:, (thi * J + j) * 512:(thi * J + j + 1) * 512],
                                   pgi[:])

            # ---- helper: per-(b) softmax over T on a [1, TOK] row ----
            def softmax_rowT(dst, src):
                for b in range(B):
                    sl = slice(b * T, (b + 1) * T)
                    nc.vector.tensor_reduce(mx[0:1, b:b + 1], src[0:1, sl],
                                            AX.X, OP.max, negate=True)
                    nc.scalar.activation(sbe[0:1, sl], src[0:1, sl], AF.Exp,
                                         bias=mx[0:1, b:b + 1], scale=1.0,
                                         accum_out=ssum[0:1, b:b + 1])
                    nc.vector.reciprocal(rs[0:1, b:b + 1], ssum[0:1, b:b + 1])
                    nc.vector.tensor_scalar_mul(dst[0:1, sl], sbe[0:1, sl],
                                                rs[0:1, b:b + 1])

            # ================= segment loop =================
            for s in range(SEGS):
                # -- recurrence --
                nc.vector.memset(TC[:], 0.0)       # c := 0 (and tg scratch)
                for t in range(T):
                    tt, thi = t % 32, t // 32
                    pg = PG[t % 2]
                    for j in range(J):
                        nc.tensor.matmul(
                            pg[32 * j:32 * j + B, :],
                            sb["I128"][:, tt * 4:tt * 4 + 4],
                            Gin[:, (thi * J + j) * 512:(thi * J + j + 1) * 512],
                            start=True, stop=(t == 0), tile_position=(0, 32 * j))
                    if t > 0:
                        for k in range(KC):
                            for j in range(J):
                                nc.tensor.matmul(
                                    pg[32 * j:32 * j + B, :],
                                    hTs[:, (t - 1) * 16 + 4 * k:(t - 1) * 16 + 4 * k + 4],
                                    sb["Whh"][:, (k * J + j) * 512:(k * J + j + 1) * 512],
                                    start=False, stop=(k == KC - 1),
                                    tile_position=(0, 32 * j))
                    # cell math
                    nc.scalar.activation(S_if[:], pg[:, 0:256], AF.Sigmoid)
                    nc.scalar.activation(TC[:, 0:128], pg[:, 256:384], AF.Tanh)
                    nc.scalar.activation(S_o[:], pg[:, 384:512], AF.Sigmoid)
                    nc.vector.tensor_mul(P2[:], S_if[:], TC[:])
                    nc.vector.tensor_add(Cn[:], P2[:, 0:128], P2[:, 128:256])
                    nc.vector.tensor_scalar_mul(OM[:], S_o[:], mask_sb[:, t:t + 1])
                    nc.scalar.activation(Tc[:], Cn[:], AF.Tanh)
                    nc.vector.tensor_mul(Hb[:], OM[:], Tc[:])
                    nc.vector.tensor_scalar_mul(TC[:, 128:256], Cn[:],
                                                mask_sb[:, t:t + 1])
                    pt = PT[t % 2]
                    nc.tensor.transpose(pt[:], Hb[:], sb["I128"][:])
                    nc.scalar.copy(
                        hTs[:, t * 16:(t + 1) * 16].rearrange("p (j c) -> p j c", j=4),
                        pt[:].rearrange("p (j c) -> p j c", j=4)[:, :, 0:4])

                # enc output DMA: o_enc[s][b,t,h] <- hTs[hh, (t,j,b)]
                for b in range(B):
                    nc.sync.dma_start(
                        o_enc[s, b].rearrange("t (j hh) -> hh t j", j=4),
                        hTs[:].rearrange("p (t j b) -> p t j b", j=4, b=B)[:, :, :, b])

                # -- boundary head --
                if s < SEGS - 1:
                    encT = hTs[:].rearrange("p (t j b) -> p j b t", j=4, b=B)
                    plb = psp.tile([1, 512], f32, tag="small_ps")
                    for m in range(4):
                        ph = pdp.tile([128, 512], f32, tag="dec_ps")
                        for k in range(KC):
                            nc.tensor.matmul(
                                ph[:], sb["hb1"][:, (k * 4 + m) * 128:(k * 4 + m + 1) * 128],
                                encT[:, k], start=(k == 0), stop=(k == KC - 1))
                        hidc = sp.tile([128, 512], f32, tag="hidc")
                        nc.scalar.activation(hidc[:], ph[:],
                                             AF.Relu, bias=sb["hb1b"][:, m:m + 1])
                        nc.tensor.matmul(plb[:], sb["hb2"][:, m:m + 1], hidc[:],
                                         start=(m == 0), stop=(m == 3),
                                         skip_group_check=True)
                    nc.scalar.activation(lb_sb[:], plb[:], AF.Identity,
                                         bias=hb2b_t[0:1, 0:1])
                    for b in range(B):
                        nc.vector.memset(lb_sb[0:1, b * T:b * T + 1], NEG_INF)
                    nc.sync.dma_start(o_lb[s].rearrange("b t -> (b t)").unsqueeze(0),
                                      lb_sb[:])
                    nc.vector.tensor_add(sbn[:], lb_sb[:],
                                         sb["gb"][0:1, s * TOK:(s + 1) * TOK])
                    softmax_rowT(sb_row, sbn)
                    sbr = sb_row
                else:
                    sbr = sb["loh"]
                nc.sync.dma_start(o_sb[s].rearrange("b t -> (b t)").unsqueeze(0), sbr[:])

                # -- readout + latent head --
                sbB = psp.tile([128, 512], f32, tag="small_ps")
                nc.tensor.matmul(sbB[:], ones128[:], sbr[0:1, :],
                                 start=True, stop=True)
                for j in range(J):
                    pr = sp.tile([128, 4 * 127], f32, tag="pr")
                    nc.vector.tensor_mul(
                        pr[:].rearrange("p (b t) -> p b t", b=B),
                        hTs[:].rearrange("p (t j b) -> p j b t", j=4, b=B)
                        [:, j, :, 0:127],
                        sbB[:].rearrange("p (b t) -> p b t", b=B)[:, :, 1:128])
                    nc.vector.tensor_reduce(
                        roT[:, 4 * j:4 * j + 4],
                        pr[:].rearrange("p (b t) -> p b t", b=B),
                        AX.X, OP.add)
                for m in range(4):
                    phz = psp.tile([128, 4], f32, tag="small_ps")
                    for k in range(KC):
                        nc.tensor.matmul(
                            phz[:], sb["hz1"][:, (k * 4 + m) * 128:(k * 4 + m + 1) * 128],
                            roT[:, 4 * k:4 * k + 4], start=(k == 0), stop=(k == KC - 1))
                    nc.scalar.activation(hzT[:, m * 4:(m + 1) * 4], phz[:],
                                         AF.Relu, bias=sb["hz1b"][:, m:m + 1])
                plz = psp.tile([B, L], f32, tag="small_ps")
                for k in range(KC):
                    nc.tensor.matmul(plz[:], hzT[:, k * 4:k * 4 + 4],
                                     sb["hz2"][:, k * L:(k + 1) * L],
                                     start=(k == 0), stop=False)
                nc.tensor.matmul(plz[:], sb["ones1"][:], sb["hz2b"][:],
                                 start=False, stop=True)
                nc.scalar.copy(lz_sb[:], plz[:])
                nc.sync.dma_start(o_lz[s][:], lz_sb[:])
                nc.vector.tensor_add(zn[:], lz_sb[:], sb["gz"][:, s * L:(s + 1) * L])
                nc.vector.tensor_reduce(nm[:], zn[:], AX.X, OP.max, negate=True)
                nc.scalar.activation(ze[:], zn[:], AF.Exp, bias=nm[:, 0:1],
                                     scale=1.0, accum_out=zs[:, 0:1])
                nc.vector.reciprocal(rz[:], zs[:])
                nc.vector.tensor_scalar_mul(z_sb[:], ze[:], rz[:, 0:1])
                nc.sync.dma_start(o_z[s][:], z_sb[:])
                pzt = psp.tile([L, B], f32, tag="small_ps")
                nc.tensor.transpose(pzt[:], z_sb[:], sb["I128"][0:B, 0:B])
                nc.scalar.copy(zT_all[s][0:L, :], pzt[:])

                # -- mask update --
                if s < SEGS - 1:
                    for b in range(B):
                        sl = slice(b * T, (b + 1) * T)
                        nc.vector.tensor_tensor_scan(
                            cum[0:1, sl], sbr[0:1, sl], zero_row[0:1, sl],
                            0.0, OP.add, OP.add)
                    nc.scalar.activation(lncum[:], cum[:], AF.Ln,
                                         bias=eps_t[0:1, 0:1])
                    nc.vector.tensor_add(logacc[:], logacc[:], lncum[:])
                    nc.scalar.activation(mask_row[:], logacc[:], AF.Exp)
                    nc.sync.dma_start(o_mask[s].rearrange("b t -> (b t)").unsqueeze(0),
                                      mask_row[:])
                    for j in range(J):
                        for b in range(B):
                            nc.sync.dma_start(mask_sb[32 * j + b:32 * j + b + 1, :],
                                              mask_row[0:1, b * T:(b + 1) * T])

            # ================= decoder (segment-independent) =================
            for l in range(L):
                w1 = sp.tile([128, 512], f32, tag="w1l")
                nc.sync.dma_start(w1[:], di["W1e"][:, l * 512:(l + 1) * 512])
                w2h = []
                for c in range(2):
                    w2c = sp.tile([128, 1024], f32, tag="w2l", name=f"w2c{c}")
                    nc.sync.dma_start(
                        w2c[:], di["W2"][:, l * 2048 + c * 1024:l * 2048 + (c + 1) * 1024])
                    w2h.append(w2c)
                h1T = sp.tile([128, 4 * 512], f32, tag="h1T", bufs=1)
                h2T = sp.tile([128, 4 * 512], f32, tag="h2T", bufs=1)
                for m in range(4):
                    p1 = pdp.tile([128, 512], f32, tag="dec_ps")
                    nc.tensor.matmul(p1[:], w1[:, m * 128:(m + 1) * 128],
                                     sb["xT"][:], start=True, stop=True)
                    nc.scalar.activation(h1T[:, m * 512:(m + 1) * 512], p1[:],
                                         AF.Tanh, bias=sb["b1e"][:, l * 4 + m:l * 4 + m + 1])
                for m in range(4):
                    p2d = pdp.tile([128, 512], f32, tag="dec_ps")
                    for k in range(KC):
                        nc.tensor.matmul(
                            p2d[:],
                            w2h[k // 2][:, ((k % 2) * 4 + m) * 128:((k % 2) * 4 + m + 1) * 128],
                            h1T[:, k * 512:(k + 1) * 512],
                            start=(k == 0), stop=(k == KC - 1))
                    nc.scalar.activation(h2T[:, m * 512:(m + 1) * 512], p2d[:],
                                         AF.Tanh, bias=sb["b2"][:, l * 4 + m:l * 4 + m + 1])
                p3 = psp.tile([A, 512], f32, tag="small_ps")
                for k in range(KC):
                    nc.tensor.matmul(p3[:], sb["W3"][:, (l * 4 + k) * A:(l * 4 + k + 1) * A],
                                     h2T[:, k * 512:(k + 1) * 512],
                                     start=(k == 0), stop=(k == KC - 1))
                outl = sp.tile([A, 512], f32, tag="outl")
                nc.scalar.activation(outl[:], p3[:],
                                     AF.Identity, bias=sb["b3"][:, l:l + 1])
                for b in range(B):
                    nc.sync.dma_start(
                        outs_dr[l, b].rearrange("(thi tt) a -> a thi tt", thi=4),
                        outl[:].rearrange("a (thi tt b) -> a thi tt b",
                                          thi=4, b=B)[:, :, :, b])

            # ---- mixing: rec[s][b,t,a] = sum_l z[b,l] * outs[l,b,t,a] ----
            for thi in range(4):
                for hb in range(2):          # pair of b values
                    OT2 = sp.tile([L, 2 * 32 * A], f32, tag="OT2", bufs=1)
                    for l in range(L):
                        # OT2[l, b'*1024 + tt*32 + a] <- outs_dr[l, 2hb+b', thi*32+tt, a]
                        nc.sync.dma_start(
                            OT2[l:l + 1, :],
                            outs_dr[l, 2 * hb:2 * hb + 2, thi * 32:(thi + 1) * 32, :]
                            .unsqueeze(0))
                    for s in range(SEGS):
                        for bl in range(2):
                            b = 2 * hb + bl
                            for c in range(2):
                                pm = psp.tile([1, 512], f32, tag="small_ps")
                                nc.tensor.matmul(
                                    pm[:], zT_all[s][0:L, b:b + 1],
                                    OT2[:, bl * 1024 + c * 512: bl * 1024 + (c + 1) * 512],
                                    start=True, stop=True)
                                nc.scalar.copy(rec_sb[0:1, c * 512:(c + 1) * 512], pm[:])
                            nc.sync.dma_start(
                                o_rec[s, b].rearrange("t a -> (t a)").unsqueeze(0)
                                [0:1, thi * 1024:(thi + 1) * 1024],
                                rec_sb[:])

    nc.compile()
    return nc


# ----------------------------------------------------------------------------
# Entry point
# ----------------------------------------------------------------------------

def kernel(**inputs):
    import sys
    for p in ("/opt/trn_rl_repo", "/root/.axon_site/_ro/trn_rl_repo"):
        if os.path.isdir(p) and p not in sys.path:
            sys.path.insert(0, p)

    gb, gz = _gumbel_noise()
    shared, hb2_b = _pack_weights(inputs)

    if "nc" not in _prog_cache:
        _prog_cache["nc"] = _build_program(hb2_b)
    nc = _prog_cache["nc"]

    in_maps = []
    for core in range(NC):
        m = dict(shared)
        m.update(_pack_core_inputs(inputs, gb, gz, core))
        in_maps.append(m)

    from concourse.bass_utils import run_bass_kernel_spmd
    res = run_bass_kernel_spmd(nc, in_maps, list(range(NC)),
                               trace=bool(os.environ.get("KERNEL_TRACE")))
    _prog_cache["exec_time_ns"] = res.exec_time_ns
    _prog_cache["profile_json"] = res.profile_json
    outs = res.results

    encs = np.concatenate([r["o_enc"] for r in outs], axis=1)
    recs = np.concatenate([r["o_rec"] for r in outs], axis=1)
    masks = np.concatenate([r["o_mask"] for r in outs], axis=1)
    lbs = np.concatenate([r["o_lb"] for r in outs], axis=1)
    sbs = np.concatenate([r["o_sb"] for r in outs], axis=1)
    lzs = np.concatenate([r["o_lz"] for r in outs], axis=1)
    zs = np.concatenate([r["o_z"] for r in outs], axis=1)
    return (encs, recs, masks, lbs, sbs, lzs, zs)


# revision 38
# speedup vs baseline: 65.9451x; 65.9451x over previous
"""Trainium2 Bass kernel for the CompILE-style model (nn_CompILE_5111011082477).

Sharding: pure data-parallel over batch B=32 across 8 cores (B=4 per core),
all parameters replicated, zero collectives.

Device program (per core) highlights:
  - All feed-forward matmuls run "activation transposed": features on
    partitions, tokens on the free dim, weights stationary.
  - Host folds the embedding into effective weights:
      Gin = x @ (embed_W @ W_ih.T[:256]) + onehot(a) @ (table @ W_ih.T[256:] + bias)
    (one-hot rows sum to 1, so all biases fold into the action table term).
  - LSTM recurrence: per step, gates land in PSUM as [(j,b) partitions,
    (gate, hh) free] via 4-way column-tiled matmuls (tile_position=(0,32j)).
    Gin is injected as the start=True matmul using an identity-selector lhsT.
    Cell math runs full-width on ACT/DVE; one PE transpose regenerates h^T.
  - Decoder (l=16 parallel MLPs) computed ONCE (it is segment-independent)
    and mixed with each segment's sample_z via small K=16 matmuls.
"""

import os
import numpy as np

B_FULL, T, D, A, H, L = 32, 128, 128, 32, 512, 16
NC = 8
B = B_FULL // NC          # 4 per core
SEGS = 4
G = 4 * H                 # 2048
HH = 128                  # h per j-slice
J = 4                     # h slices
KC = 4                    # contraction chunks of H
TOK = B * T               # 512 per core
NEG_INF = -1e30
EPS = 1e-17

_prog_cache = {}


def last_exec_time_ns():
    return _prog_cache.get("exec_time_ns")


# ----------------------------------------------------------------------------
# Host-side packing
# ----------------------------------------------------------------------------

def _gumbel_noise():
    """Bit-identical gumbel noise to reference (key 42, CPU)."""
    import jax
    cpu = jax.devices("cpu")[0]
    with jax.default_device(cpu):
        nkey = jax.random.key(42)
        gb = np.stack([
            np.asarray(jax.random.gumbel(jax.random.fold_in(nkey, 2 * s),
                                         (B_FULL, T), jax.numpy.float32))
            for s in range(SEGS - 1)])            # [3, 32, 128]
        gz = np.stack([
            np.asarray(jax.random.gumbel(jax.random.fold_in(nkey, 2 * s + 1),
                                         (B_FULL, L), jax.numpy.float32))
            for s in range(SEGS)])                # [4, 32, 16]
    return gb, gz


def _pack_weights(inp):
    """Build all shared (replicated) device tensors. float64 intermediates."""
    f = np.float32
    embed_W = inp["embed_W"].astype(np.float64)        # [128, 256]
    embed_b = inp["embed_b"].astype(np.float64)        # [256]
    table = inp["embed_action_table"].astype(np.float64)  # [32, 256]
    W_ih = inp["W_ih"].astype(np.float64)              # [2048, 512]
    W_hh = inp["W_hh"].astype(np.float64)              # [2048, 512]
    b_ih = inp["b_ih"].astype(np.float64)
    b_hh = inp["b_hh"].astype(np.float64)

    WihT = W_ih.T                                      # [512, 2048]
    M1 = embed_W @ WihT[:256]                          # [128, 2048] gate idx g*512+h'
    bias_g = b_ih + b_hh + embed_b @ WihT[:256]        # [2048]
    M2p = table @ WihT[256:] + bias_g[None, :]         # [32, 2048]

    def gate_reindex(M):  # [..., g*512 + j*128 + hh] -> [..., (j*4+g)*128 + hh]
        Mr = M.reshape(M.shape[0], 4, J, HH)           # [in, g, j, hh]
        return np.ascontiguousarray(Mr.transpose(0, 2, 1, 3).reshape(M.shape[0], G))

    M1_dev = gate_reindex(M1).astype(f)
    M2p_dev = gate_reindex(M2p).astype(f)

    # Whh_dev[kk, ((k*4+j)*4+g)*128+hh] = W_hh[g*512+j*128+hh, k*128+kk]
    Whh = W_hh.reshape(4, J, HH, KC, 128)              # [g, j, hh, k, kk]
    Whh_dev = np.ascontiguousarray(
        Whh.transpose(4, 3, 1, 0, 2).reshape(128, KC * J * 4 * HH)).astype(f)

    def chunk2(Wmat):  # [512, 512] -> [128, (k*4+m)*128+mm]
        Wr = Wmat.reshape(KC, 128, 4, 128)             # [k, kk, m, mm]
        return np.ascontiguousarray(Wr.transpose(1, 0, 2, 3).reshape(128, 2048))

    hb1_dev = chunk2(inp["hb1_W"].astype(np.float64)).astype(f)
    hz1_dev = chunk2(inp["hz1_W"].astype(np.float64)).astype(f)
    hb1b_dev = np.ascontiguousarray(
        inp["hb1_b"].astype(np.float64).reshape(4, 128).T).astype(f)   # [mm, m]
    hz1b_dev = np.ascontiguousarray(
        inp["hz1_b"].astype(np.float64).reshape(4, 128).T).astype(f)
    hb2_dev = np.ascontiguousarray(
        inp["hb2_W"].astype(np.float64).reshape(KC, 128).T).astype(f)  # [kk, k]
    hb2_b = float(inp["hb2_b"][0])
    hz2 = inp["hz2_W"].astype(np.float64).reshape(KC, 128, L)          # [k, kk, l]
    hz2_dev = np.ascontiguousarray(hz2.transpose(1, 0, 2).reshape(128, KC * L)).astype(f)
    hz2b_row = inp["hz2_b"].astype(f).reshape(1, L)

    dec1 = inp["dec1_W"].astype(np.float64)            # [L, 256, 512]
    dec1b = inp["dec1_b"].astype(np.float64)           # [L, 512]
    W1e = np.einsum("dh,lhk->ldk", embed_W, dec1)      # [L, 128, 512]
    b1e = dec1b + np.einsum("h,lhk->lk", embed_b, dec1)
    # W1e_dev[dd, (l*4+m)*128+mm]
    W1e_dev = np.ascontiguousarray(
        W1e.reshape(L, 128, 4, 128).transpose(1, 0, 2, 3).reshape(128, L * 512)).astype(f)
    b1e_dev = np.ascontiguousarray(
        b1e.reshape(L, 4, 128).transpose(2, 0, 1).reshape(128, L * 4)).astype(f)

    dec2 = inp["dec2_W"].astype(np.float64)            # [L, 512, 512]
    # W2_dev[kk, ((l*4+k)*4+m)*128+mm]
    W2_dev = np.ascontiguousarray(
        dec2.reshape(L, KC, 128, 4, 128).transpose(2, 0, 1, 3, 4)
        .reshape(128, L * 2048)).astype(f)
    b2_dev = np.ascontiguousarray(
        inp["dec2_b"].astype(np.float64).reshape(L, 4, 128)
        .transpose(2, 0, 1).reshape(128, L * 4)).astype(f)

    dec3 = inp["dec3_W"].astype(np.float64)            # [L, 512, 32]
    W3_dev = np.ascontiguousarray(
        dec3.reshape(L, KC, 128, A).transpose(2, 0, 1, 3).reshape(128, L * KC * A)).astype(f)
    b3_dev = np.ascontiguousarray(
        inp["dec3_b"].astype(np.float64).T).astype(f)  # [a, l]

    I128 = np.eye(128, dtype=f)
    ones1 = np.ones((1, 4), dtype=f)

    return dict(M1=M1_dev, M2p=M2p_dev, Whh=Whh_dev, hb1=hb1_dev, hb1b=hb1b_dev,
                hb2=hb2_dev, hz1=hz1_dev, hz1b=hz1b_dev, hz2=hz2_dev,
                hz2b=hz2b_row, W1e=W1e_dev, b1e=b1e_dev, W2=W2_dev, b2=b2_dev,
                W3=W3_dev, b3=b3_dev, I128=I128, ones1=ones1), hb2_b


def _pack_core_inputs(inp, gb, gz, core):
    """Per-core activation tensors. tok col = t_hi*128 + tt*4 + b."""
    f = np.float32
    b0 = core * B
    x = np.asarray(inp["inputs"][b0:b0 + B], dtype=f)          # [4, 128, 128]
    act = np.asarray(inp["actions"][b0:b0 + B]).astype(np.int64)
    lens = np.asarray(inp["lengths"][b0:b0 + B]).astype(np.int64)

    # xT[d, t_hi*128 + tt*4 + b] = x[b, t_hi*32+tt, d]
    xr = x.reshape(B, 4, 32, D)                                # [b, t_hi, tt, d]
    xT = np.ascontiguousarray(xr.transpose(3, 1, 2, 0).reshape(D, TOK)).astype(f)
    aoh_full = np.zeros((A, B, T), dtype=f)
    for b in range(B):
        aoh_full[act[b], b, np.arange(T)] = 1.0
    aohr = aoh_full.reshape(A, B, 4, 32)                       # [a, b, t_hi, tt]
    aoh = np.ascontiguousarray(aohr.transpose(0, 2, 3, 1).reshape(A, TOK)).astype(f)

    gb_c = gb[:, b0:b0 + B, :]                                 # [3, 4, 128]
    gb_dev = np.ascontiguousarray(gb_c.reshape(1, 3 * B * T)).astype(f)
    gz_c = gz[:, b0:b0 + B, :]                                 # [4, 4, 16]
    gz_dev = np.ascontiguousarray(gz_c.transpose(1, 0, 2).reshape(B, SEGS * L)).astype(f)

    loh = np.zeros((1, B * T), dtype=f)
    for b in range(B):
        loh[0, b * T + (int(lens[b]) - 1)] = 1.0
    return dict(xT=xT, aoh=aoh, gb=gb_dev, gz=gz_dev, loh=loh)


# ----------------------------------------------------------------------------
# Device program
# ----------------------------------------------------------------------------

def _build_program(hb2_b):
    import concourse.bass as bass
    import concourse.mybir as mybir
    from concourse import bacc, tile

    f32 = mybir.dt.float32
    AF = mybir.ActivationFunctionType
    OP = mybir.AluOpType
    AX = mybir.AxisListType

    nc = bacc.Bacc(None, target_bir_lowering=False, debug=False)

    # ---- DRAM I/O ----
    di = {}
    def d_in(name, shape):
        di[name] = nc.dram_tensor(name, list(shape), f32, kind="ExternalInput")
        return di[name]

    for name, shape in [
        ("xT", (D, TOK)), ("aoh", (A, TOK)), ("M1", (D, G)), ("M2p", (A, G)),
        ("Whh", (128, KC * J * 4 * HH)), ("hb1", (128, 2048)), ("hb1b", (128, 4)),
        ("hb2", (128, 4)), ("hz1", (128, 2048)), ("hz1b", (128, 4)),
        ("hz2", (128, KC * L)), ("hz2b", (1, L)), ("W1e", (128, L * 512)),
        ("b1e", (128, L * 4)), ("W2", (128, L * 2048)), ("b2", (128, L * 4)),
        ("W3", (128, L * KC * A)), ("b3", (A, L)), ("I128", (128, 128)),
        ("ones1", (1, 4)), ("gb", (1, 3 * B * T)), ("gz", (B, SEGS * L)),
        ("loh", (1, B * T)),
    ]:
        d_in(name, shape)

    outs_dr = nc.dram_tensor("outs_dr", [L, B, T, A], f32)
    o_enc = nc.dram_tensor("o_enc", [SEGS, B, T, H], f32, kind="ExternalOutput")
    o_rec = nc.dram_tensor("o_rec", [SEGS, B, T, A], f32, kind="ExternalOutput")
    o_mask = nc.dram_tensor("o_mask", [SEGS - 1, B, T], f32, kind="ExternalOutput")
    o_lb = nc.dram_tensor("o_lb", [SEGS - 1, B, T], f32, kind="ExternalOutput")
    o_sb = nc.dram_tensor("o_sb", [SEGS, B, T], f32, kind="ExternalOutput")
    o_lz = nc.dram_tensor("o_lz", [SEGS, B, L], f32, kind="ExternalOutput")
    o_z = nc.dram_tensor("o_z", [SEGS, B, L], f32, kind="ExternalOutput")

    with tile.TileContext(nc) as tc:
        with (
            tc.tile_pool(name="w", bufs=1) as wp,
            tc.tile_pool(name="stream", bufs=2) as sp,
            tc.tile_pool(name="pg", bufs=1, space="PSUM") as pgp,
            tc.tile_pool(name="pt", bufs=1, space="PSUM") as ptp,
            tc.tile_pool(name="pd", bufs=2, space="PSUM") as pdp,
            tc.tile_pool(name="ps", bufs=2, space="PSUM") as psp,
        ):
            # ---- resident SBUF tiles + input DMAs ----
            sb = {}
            for name in ["xT", "aoh", "M1", "M2p", "Whh", "hb1", "hb1b", "hb2",
                         "hz1", "hz1b", "hz2", "hz2b", "b1e", "b2", "W3", "b3",
                         "I128", "ones1", "gb", "gz", "loh"]:
                t = wp.tile(list(di[name].shape), f32, tag=name)
                nc.sync.dma_start(t[:], di[name][:])
                sb[name] = t

            Gin = wp.tile([128, KC * J * 4 * HH], f32, tag="Gin")   # [ (tt,b), (t_hi,j,g,hh) ]
            hTs = wp.tile([128, T * 16], f32, tag="hTs")            # [hh, (t,j,b)]
            S_if = wp.tile([128, 256], f32, tag="S_if")
            S_o = wp.tile([128, 128], f32, tag="S_o")
            TC = wp.tile([128, 256], f32, tag="TC")                 # [tg | c]
            P2 = wp.tile([128, 256], f32, tag="P2")
            Cn = wp.tile([128, 128], f32, tag="Cn")
            Tc = wp.tile([128, 128], f32, tag="Tc")
            OM = wp.tile([128, 128], f32, tag="OM")
            Hb = wp.tile([128, 128], f32, tag="Hb")
            mask_sb = wp.tile([128, T], f32, tag="mask_sb")
            ones128 = wp.tile([1, 128], f32, tag="ones128")
            roT = wp.tile([128, 16], f32, tag="roT")
            hzT = wp.tile([128, 16], f32, tag="hzT")
            lb_sb = wp.tile([1, TOK], f32, tag="lb_sb")
            sbn = wp.tile([1, TOK], f32, tag="sbn")
            sbe = wp.tile([1, TOK], f32, tag="sbe")
            sb_row = wp.tile([1, TOK], f32, tag="sb_row")
            cum = wp.tile([1, TOK], f32, tag="cum")
            lncum = wp.tile([1, TOK], f32, tag="lncum")
            logacc = wp.tile([1, TOK], f32, tag="logacc")
            mask_row = wp.tile([1, TOK], f32, tag="mask_row")
            zero_row = wp.tile([1, TOK], f32, tag="zero_row")
            mx = wp.tile([1, B], f32, tag="mx")
            ssum = wp.tile([1, B], f32, tag="ssum")
            rs = wp.tile([1, B], f32, tag="rs")
            lz_sb = wp.tile([B, L], f32, tag="lz_sb")
            zn = wp.tile([B, L], f32, tag="zn")
            ze = wp.tile([B, L], f32, tag="ze")
            z_sb = wp.tile([B, L], f32, tag="z_sb")
            nm = wp.tile([B, 1], f32, tag="nm")
            zs = wp.tile([B, 1], f32, tag="zs")
            rz = wp.tile([B, 1], f32, tag="rz")
            rec_sb = wp.tile([1, 1024], f32, tag="rec_sb")
            zT_all = [wp.tile([128, B], f32, tag=f"zT{s}", name=f"zT{s}")
                      for s in range(SEGS)]

            nc.vector.memset(ones128[:], 1.0)
            eps_t = wp.tile([128, 1], f32, tag="eps_t")
            hb2b_t = wp.tile([128, 1], f32, tag="hb2b_t")
            nc.vector.memset(eps_t[:], EPS)
            nc.vector.memset(hb2b_t[:], hb2_b)
            nc.vector.memset(mask_sb[:], 1.0)
            nc.vector.memset(logacc[:], 0.0)
            nc.vector.memset(zero_row[:], 0.0)
            nc.vector.memset(Hb[:], 0.0)

            # persistent psum tiles (ping-pong)
            PG = [pgp.tile([128, 512], f32, tag=f"pg{i}", name=f"pg{i}")
                  for i in range(2)]
            PT = [ptp.tile([128, 128], f32, tag=f"pt{i}", name=f"pt{i}")
                  for i in range(2)]
            nc.vector.memset(PG[0][:], 0.0)
            nc.vector.memset(PG[1][:], 0.0)

            # ---- Gin precompute ----
            for thi in range(4):
                for j in range(J):
                    pgi = pdp.tile([128, 512], f32, tag="dec_ps")
                    nc.tensor.matmul(pgi[:], sb["xT"][:, thi * 128:(thi + 1) * 128],
                                     sb["M1"][:, j * 512:(j + 1) * 512],
                                     start=True, stop=False)
                    nc.tensor.matmul(pgi[:], sb["aoh"][:, thi * 128:(thi + 1) * 128],
                                     sb["M2p"][:, j * 512:(j + 1) * 512],
                                     start=False, stop=True)
                    nc.scalar.copy(Gin[:, (thi * J + j) * 512:(thi * J + j + 1) * 512],
                                   pgi[:])

            # ---- helper: per-(b) softmax over T on a [1, TOK] row ----
            def softmax_rowT(dst, src):
                for b in range(B):
                    sl = slice(b * T, (b + 1) * T)
                    nc.vector.tensor_reduce(mx[0:1, b:b + 1], src[0:1, sl],
                                            AX.X, OP.max, negate=True)
                    nc.scalar.activation(sbe[0:1, sl], src[0:1, sl], AF.Exp,
                                         bias=mx[0:1, b:b + 1], scale=1.0,
                                         accum_out=ssum[0:1, b:b + 1])
                    nc.vector.reciprocal(rs[0:1, b:b + 1], ssum[0:1, b:b + 1])
                    nc.vector.tensor_scalar_mul(dst[0:1, sl], sbe[0:1, sl],
                                                rs[0:1, b:b + 1])

            # ================= segment loop =================
            for s in range(SEGS):
                # -- recurrence --
                nc.vector.memset(TC[:], 0.0)       # c := 0 (and tg scratch)
                for t in range(T):
                    tt, thi = t % 32, t // 32
                    pg = PG[t % 2]
                    for j in range(J):
                        nc.tensor.matmul(
                            pg[32 * j:32 * j + B, :],
                            sb["I128"][:, tt * 4:tt * 4 + 4],
                            Gin[:, (thi * J + j) * 512:(thi * J + j + 1) * 512],
                            start=True, stop=(t == 0), tile_position=(0, 32 * j))
                    if t > 0:
                        for k in range(KC):
                            for j in range(J):
                                nc.tensor.matmul(
                                    pg[32 * j:32 * j + B, :],
                                    hTs[:, (t - 1) * 16 + 4 * k:(t - 1) * 16 + 4 * k + 4],
                                    sb["Whh"][:, (k * J + j) * 512:(k * J + j + 1) * 512],
                                    start=False, stop=(k == KC - 1),
                                    tile_position=(0, 32 * j))
                    # cell math
                    nc.scalar.activation(S_if[:], pg[:, 0:256], AF.Sigmoid)
                    nc.scalar.activation(TC[:, 0:128], pg[:, 256:384], AF.Tanh)
                    nc.scalar.activation(S_o[:], pg[:, 384:512], AF.Sigmoid)
                    nc.vector.tensor_mul(P2[:], S_if[:], TC[:])
                    nc.vector.tensor_add(Cn[:], P2[:, 0:128], P2[:, 128:256])
                    nc.vector.tensor_scalar_mul(OM[:], S_o[:], mask_sb[:, t:t + 1])
                    nc.scalar.activation(Tc[:], Cn[:], AF.Tanh)
                    nc.vector.tensor_mul(Hb[:], OM[:], Tc[:])
                    nc.vector.tensor_scalar_mul(TC[:, 128:256], Cn[:],
                                                mask_sb[:, t:t + 1])
                    pt = PT[t % 2]
                    nc.tensor.transpose(pt[:], Hb[:], sb["I128"][:])
                    nc.scalar.copy(
                        hTs[:, t * 16:(t + 1) * 16].rearrange("p (j c) -> p j c", j=4),
                        pt[:].rearrange("p (j c) -> p j c", j=4)[:, :, 0:4])

                # enc output DMA: o_enc[s][b,t,h] <- hTs[hh, (t,j,b)]
                for b in range(B):
                    nc.sync.dma_start(
                        o_enc[s, b].rearrange("t (j hh) -> hh t j", j=4),
                        hTs[:].rearrange("p (t j b) -> p t j b", j=4, b=B)[:, :, :, b])

                # -- boundary head --
                if s < SEGS - 1:
                    encT = hTs[:].rearrange("p (t j b) -> p j b t", j=4, b=B)
                    plb = psp.tile([1, 512], f32, tag="small_ps")
                    for m in range(4):
                        ph = pdp.tile([128, 512], f32, tag="dec_ps")
                        for k in range(KC):
                            nc.tensor.matmul(
                                ph[:], sb["hb1"][:, (k * 4 + m) * 128:(k * 4 + m + 1) * 128],
                                encT[:, k], start=(k == 0), stop=(k == KC - 1))
                        hidc = sp.tile([128, 512], f32, tag="hidc")
                        nc.scalar.activation(hidc[:], ph[:],
                                             AF.Relu, bias=sb["hb1b"][:, m:m + 1])
                        nc.tensor.matmul(plb[:], sb["hb2"][:, m:m + 1], hidc[:],
                                         start=(m == 0), stop=(m == 3),
                                         skip_group_check=True)
                    nc.scalar.activation(lb_sb[:], plb[:], AF.Identity,
                                         bias=hb2b_t[0:1, 0:1])
                    for b in range(B):
                        nc.vector.memset(lb_sb[0:1, b * T:b * T + 1], NEG_INF)
                    nc.sync.dma_start(o_lb[s].rearrange("b t -> (b t)").unsqueeze(0),
                                      lb_sb[:])
                    nc.vector.tensor_add(sbn[:], lb_sb[:],
                                         sb["gb"][0:1, s * TOK:(s + 1) * TOK])
                    softmax_rowT(sb_row, sbn)
                    sbr = sb_row
                else:
                    sbr = sb["loh"]
                nc.sync.dma_start(o_sb[s].rearrange("b t -> (b t)").unsqueeze(0), sbr[:])

                # -- readout + latent head --
                sbB = psp.tile([128, 512], f32, tag="small_ps")
                nc.tensor.matmul(sbB[:], ones128[:], sbr[0:1, :],
                                 start=True, stop=True)
                for j in range(J):
                    pr = sp.tile([128, 4 * 127], f32, tag="pr")
                    nc.vector.tensor_mul(
                        pr[:].rearrange("p (b t) -> p b t", b=B),
                        hTs[:].rearrange("p (t j b) -> p j b t", j=4, b=B)
                        [:, j, :, 0:127],
                        sbB[:].rearrange("p (b t) -> p b t", b=B)[:, :, 1:128])
                    nc.vector.tensor_reduce(
                        roT[:, 4 * j:4 * j + 4],
                        pr[:].rearrange("p (b t) -> p b t", b=B),
                        AX.X, OP.add)
                for m in range(4):
                    phz = psp.tile([128, 4], f32, tag="small_ps")
                    for k in range(KC):
                        nc.tensor.matmul(
                            phz[:], sb["hz1"][:, (k * 4 + m) * 128:(k * 4 + m + 1) * 128],
                            roT[:, 4 * k:4 * k + 4], start=(k == 0), stop=(k == KC - 1))
                    nc.scalar.activation(hzT[:, m * 4:(m + 1) * 4], phz[:],
                                         AF.Relu, bias=sb["hz1b"][:, m:m + 1])
                plz = psp.tile([B, L], f32, tag="small_ps")
                for k in range(KC):
                    nc.tensor.matmul(plz[:], hzT[:, k * 4:k * 4 + 4],
                                     sb["hz2"][:, k * L:(k + 1) * L],
                                     start=(k == 0), stop=False)
                nc.tensor.matmul(plz[:], sb["ones1"][:], sb["hz2b"][:],
                                 start=False, stop=True)
                nc.scalar.copy(lz_sb[:], plz[:])
                nc.sync.dma_start(o_lz[s][:], lz_sb[:])
                nc.vector.tensor_add(zn[:], lz_sb[:], sb["gz"][:, s * L:(s + 1) * L])
                nc.vector.tensor_reduce(nm[:], zn[:], AX.X, OP.max, negate=True)
                nc.scalar.activation(ze[:], zn[:], AF.Exp, bias=nm[:, 0:1],
                                     scale=1.0, accum_out=zs[:, 0:1])
                nc.vector.reciprocal(rz[:], zs[:])
                nc.vector.tensor_scalar_mul(z_sb[:], ze[:], rz[:, 0:1])
                nc.sync.dma_start(o_z[s][:], z_sb[:])
                pzt = psp.tile([L, B], f32, tag="small_ps")
                nc.tensor.transpose(pzt[:], z_sb[:], sb["I128"][0:B, 0:B])
                nc.scalar.copy(zT_all[s][0:L, :], pzt[:])

                # -- mask update --
                if s < SEGS - 1:
                    for b in range(B):
                        sl = slice(b * T, (b + 1) * T)
                        nc.vector.tensor_tensor_scan(
                            cum[0:1, sl], sbr[0:1, sl], zero_row[0:1, sl],
                            0.0, OP.add, OP.add)
                    nc.scalar.activation(lncum[:], cum[:], AF.Ln,
                                         bias=eps_t[0:1, 0:1])
                    nc.vector.tensor_add(logacc[:], logacc[:], lncum[:])
                    nc.scalar.activation(mask_row[:], logacc[:], AF.Exp)
                    nc.sync.dma_start(o_mask[s].rearrange("b t -> (b t)").unsqueeze(0),
                                      mask_row[:])
                    for j in range(J):
                        for b in range(B):
                            nc.sync.dma_start(mask_sb[32 * j + b:32 * j + b + 1, :],
                                              mask_row[0:1, b * T:(b + 1) * T])

            # ================= decoder (segment-independent) =================
            for l in range(L):
                w1 = sp.tile([128, 512], f32, tag="w1l")
                nc.sync.dma_start(w1[:], di["W1e"][:, l * 512:(l + 1) * 512])
                w2h = []
                for c in range(2):
                    w2c = sp.tile([128, 1024], f32, tag="w2l", name=f"w2c{c}")
                    nc.sync.dma_start(
                        w2c[:], di["W2"][:, l * 2048 + c * 1024:l * 2048 + (c + 1) * 1024])
                    w2h.append(w2c)
                h1T = sp.tile([128, 4 * 512], f32, tag="h1T", bufs=1)
                h2T = sp.tile([128, 4 * 512], f32, tag="h2T", bufs=1)
                for m in range(4):
                    p1 = pdp.tile([128, 512], f32, tag="dec_ps")
                    nc.tensor.matmul(p1[:], w1[:, m * 128:(m + 1) * 128],
                                     sb["xT"][:], start=True, stop=True)
                    nc.scalar.activation(h1T[:, m * 512:(m + 1) * 512], p1[:],
                                         AF.Tanh, bias=sb["b1e"][:, l * 4 + m:l * 4 + m + 1])
                for m in range(4):
                    p2d = pdp.tile([128, 512], f32, tag="dec_ps")
                    for k in range(KC):
                        nc.tensor.matmul(
                            p2d[:],
                            w2h[k // 2][:, ((k % 2) * 4 + m) * 128:((k % 2) * 4 + m + 1) * 128],
                            h1T[:, k * 512:(k + 1) * 512],
                            start=(k == 0), stop=(k == KC - 1))
                    nc.scalar.activation(h2T[:, m * 512:(m + 1) * 512], p2d[:],
                                         AF.Tanh, bias=sb["b2"][:, l * 4 + m:l * 4 + m + 1])
                p3 = psp.tile([A, 512], f32, tag="small_ps")
                for k in range(KC):
                    nc.tensor.matmul(p3[:], sb["W3"][:, (l * 4 + k) * A:(l * 4 + k + 1) * A],
                                     h2T[:, k * 512:(k + 1) * 512],
                                     start=(k == 0), stop=(k == KC - 1))
                outl = sp.tile([A, 512], f32, tag="outl")
                nc.scalar.activation(outl[:], p3[:],
                                     AF.Identity, bias=sb["b3"][:, l:l + 1])
                for b in range(B):
                    nc.sync.dma_start(
                        outs_dr[l, b].rearrange("(thi tt) a -> a thi tt", thi=4),
                        outl[:].rearrange("a (thi tt b) -> a thi tt b",
                                          thi=4, b=B)[:, :, :, b])

            # ---- mixing: rec[s][b,t,a] = sum_l z[b,l] * outs[l,b,t,a] ----
            for thi in range(4):
                for hb in range(2):          # pair of b values
                    OT2 = sp.tile([L, 2 * 32 * A], f32, tag="OT2", bufs=1)
                    for l in range(L):
                        # OT2[l, b'*1024 + tt*32 + a] <- outs_dr[l, 2hb+b', thi*32+tt, a]
                        nc.sync.dma_start(
                            OT2[l:l + 1, :],
                            outs_dr[l, 2 * hb:2 * hb + 2, thi * 32:(thi + 1) * 32, :]
                            .unsqueeze(0))
                    for s in range(SEGS):
                        for bl in range(2):
                            b = 2 * hb + bl
                            for c in range(2):
                                pm = psp.tile([1, 512], f32, tag="small_ps")
                                nc.tensor.matmul(
                                    pm[:], zT_all[s][0:L, b:b + 1],
                                    OT2[:, bl * 1024 + c * 512: bl * 1024 + (c + 1) * 512],
                                    start=True, stop=True)
                                nc.scalar.copy(rec_sb[0:1, c * 512:(c + 1) * 512], pm[:])
                            nc.sync.dma_start(
                                o_rec[s, b].rearrange("t a -> (t a)").unsqueeze(0)
                                [0:1, thi * 1024:(thi + 1) * 1024],
                                rec_sb[:])

    nc.compile()
    return nc


# ----------------------------------------------------------------------------
# Entry point
# ----------------------------------------------------------------------------

def benchmark(inputs, iters=10):
    """Time repeated device executions (inputs resident, no donation).

    Returns (best_ns, mean_ns). Requires all outputs fully written by the
    kernel (true here), since output buffers are not pre-zeroed per call.
    """
    import sys, time
    for p in ("/opt/trn_rl_repo", "/root/.axon_site/_ro/trn_rl_repo"):
        if os.path.isdir(p) and p not in sys.path:
            sys.path.insert(0, p)
    import jax
    import numpy as np
    from jax.sharding import Mesh, PartitionSpec
    from jax.experimental.shard_map import shard_map
    import concourse.mybir as mybir
    from concourse import bass2jax
    from concourse.bass2jax import _bass_exec_p, partition_id_tensor

    gb, gz = _gumbel_noise()
    shared, hb2_b = _pack_weights(inputs)
    if "nc" not in _prog_cache:
        _prog_cache["nc"] = _build_program(hb2_b)
    nc = _prog_cache["nc"]
    bass2jax.install_neuronx_cc_hook()

    in_maps = []
    for core in range(NC):
        m = dict(shared)
        m.update(_pack_core_inputs(inputs, gb, gz, core))
        in_maps.append(m)

    partition_name = nc.partition_id_tensor.name if nc.partition_id_tensor else None
    in_names, out_names, out_avals, zero_outs = [], [], [], []
    for alloc in nc.m.functions[0].allocations:
        if not isinstance(alloc, mybir.MemoryLocationSet):
            continue
        name = alloc.memorylocations[0].name
        if alloc.kind == "ExternalInput":
            if name != partition_name:
                in_names.append(name)
        elif alloc.kind == "ExternalOutput":
            shape = tuple(alloc.tensor_shape)
            dtype = mybir.dt.np(alloc.dtype)
            out_names.append(name)
            out_avals.append(jax.core.ShapedArray(shape, dtype))
            zero_outs.append(np.zeros(shape, dtype))
    n_params = len(in_names)
    in_names_all = in_names + out_names
    if partition_name is not None:
        in_names_all = in_names_all + [partition_name]

    def _body(*args):
        operands = list(args)
        if partition_name is not None:
            operands.append(partition_id_tensor())
        outs = _bass_exec_p.bind(
            *operands,
            out_avals=tuple(out_avals),
            in_names=tuple(in_names_all),
            out_names=tuple(out_names),
            lowering_input_output_aliases=(),
            sim_require_finite=True,
            sim_require_nnan=True,
            nc=nc,
        )
        return tuple(outs)

    devices = jax.devices()[:NC]
    mesh = Mesh(np.asarray(devices), ("core",))
    nin = n_params + len(out_names)
    sharded = jax.jit(
        shard_map(_body, mesh=mesh, in_specs=(PartitionSpec("core"),) * nin,
                  out_specs=(PartitionSpec("core"),) * len(out_names),
                  check_rep=False),
        keep_unused=True)
    concat_in = [np.concatenate([np.asarray(in_maps[c][nm]) for c in range(NC)], axis=0)
                 for nm in in_names]
    concat_in += [np.concatenate([z] * NC, axis=0) for z in zero_outs]
    dev_in = jax.device_put(concat_in)
    r = sharded(*dev_in)
    jax.block_until_ready(r)
    times = []
    for _ in range(iters):
        t0 = time.perf_counter()
        r = sharded(*dev_in)
        jax.block_until_ready(r)
        times.append(time.perf_counter() - t0)
    return int(min(times) * 1e9), int(np.mean(times) * 1e9)


def kernel(**inputs):
    import sys
    for p in ("/opt/trn_rl_repo", "/root/.axon_site/_ro/trn_rl_repo"):
        if os.path.isdir(p) and p not in sys.path:
            sys.path.insert(0, p)

    gb, gz = _gumbel_noise()
    shared, hb2_b = _pack_weights(inputs)

    if "nc" not in _prog_cache:
        _prog_cache["nc"] = _build_program(hb2_b)
    nc = _prog_cache["nc"]

    in_maps = []
    for core in range(NC):
        m = dict(shared)
        m.update(_pack_core_inputs(inputs, gb, gz, core))
        in_maps.append(m)

    from concourse.bass_utils import run_bass_kernel_spmd
    res = run_bass_kernel_spmd(nc, in_maps, list(range(NC)),
                               trace=bool(os.environ.get("KERNEL_TRACE")))
    _prog_cache["exec_time_ns"] = res.exec_time_ns
    _prog_cache["profile_json"] = res.profile_json
    outs = res.results

    encs = np.concatenate([r["o_enc"] for r in outs], axis=1)
    recs = np.concatenate([r["o_rec"] for r in outs], axis=1)
    masks = np.concatenate([r["o_mask"] for r in outs], axis=1)
    lbs = np.concatenate([r["o_lb"] for r in outs], axis=1)
    sbs = np.concatenate([r["o_sb"] for r in outs], axis=1)
    lzs = np.concatenate([r["o_lz"] for r in outs], axis=1)
    zs = np.concatenate([r["o_z"] for r in outs], axis=1)
    return (encs, recs, masks, lbs, sbs, lzs, zs)


# revision 50
# speedup vs baseline: 66.1533x; 1.0032x over previous
"""Trainium2 Bass kernel for the CompILE-style model (nn_CompILE_5111011082477).

Sharding: pure data-parallel over batch B=32 across 8 cores (B=4 per core),
all parameters replicated, zero collectives.

Device program (per core) highlights:
  - All feed-forward matmuls run "activation transposed": features on
    partitions, tokens on the free dim, weights stationary.
  - Host folds the embedding into effective weights:
      Gin = x @ (embed_W @ W_ih.T[:256]) + onehot(a) @ (table @ W_ih.T[256:] + bias)
    (one-hot rows sum to 1, so all biases fold into the action table term).
  - LSTM recurrence: per step, gates land in PSUM as [(j,b) partitions,
    (gate, hh) free] via 4-way column-tiled matmuls (tile_position=(0,32j)).
    Gin is injected as the start=True matmul using an identity-selector lhsT.
    Cell math runs full-width on ACT/DVE; one PE transpose regenerates h^T.
  - Decoder (l=16 parallel MLPs) computed ONCE (it is segment-independent)
    and mixed with each segment's sample_z via small K=16 matmuls.
"""

import os
import numpy as np

B_FULL, T, D, A, H, L = 32, 128, 128, 32, 512, 16
NC = 8
B = B_FULL // NC          # 4 per core
SEGS = 4
G = 4 * H                 # 2048
HH = 128                  # h per j-slice
J = 4                     # h slices
KC = 4                    # contraction chunks of H
TOK = B * T               # 512 per core
NEG_INF = -1e30
EPS = 1e-17

_prog_cache = {}


def last_exec_time_ns():
    return _prog_cache.get("exec_time_ns")


# ----------------------------------------------------------------------------
# Host-side packing
# ----------------------------------------------------------------------------

def _gumbel_noise():
    """Bit-identical gumbel noise to reference (key 42, CPU)."""
    import jax
    cpu = jax.devices("cpu")[0]
    with jax.default_device(cpu):
        nkey = jax.random.key(42)
        gb = np.stack([
            np.asarray(jax.random.gumbel(jax.random.fold_in(nkey, 2 * s),
                                         (B_FULL, T), jax.numpy.float32))
            for s in range(SEGS - 1)])            # [3, 32, 128]
        gz = np.stack([
            np.asarray(jax.random.gumbel(jax.random.fold_in(nkey, 2 * s + 1),
                                         (B_FULL, L), jax.numpy.float32))
            for s in range(SEGS)])                # [4, 32, 16]
    return gb, gz


def _pack_weights(inp):
    """Build all shared (replicated) device tensors. float64 intermediates."""
    f = np.float32
    embed_W = inp["embed_W"].astype(np.float64)        # [128, 256]
    embed_b = inp["embed_b"].astype(np.float64)        # [256]
    table = inp["embed_action_table"].astype(np.float64)  # [32, 256]
    W_ih = inp["W_ih"].astype(np.float64)              # [2048, 512]
    W_hh = inp["W_hh"].astype(np.float64)              # [2048, 512]
    b_ih = inp["b_ih"].astype(np.float64)
    b_hh = inp["b_hh"].astype(np.float64)

    WihT = W_ih.T                                      # [512, 2048]
    M1 = embed_W @ WihT[:256]                          # [128, 2048] gate idx g*512+h'
    bias_g = b_ih + b_hh + embed_b @ WihT[:256]        # [2048]
    M2p = table @ WihT[256:] + bias_g[None, :]         # [32, 2048]

    def gate_reindex(M):  # [..., g*512 + j*128 + hh] -> [..., (j*4+g)*128 + hh]
        Mr = M.reshape(M.shape[0], 4, J, HH)           # [in, g, j, hh]
        return np.ascontiguousarray(Mr.transpose(0, 2, 1, 3).reshape(M.shape[0], G))

    M1_dev = gate_reindex(M1).astype(f)
    M2p_dev = gate_reindex(M2p).astype(f)

    # Whh_dev[kk, ((k*4+j)*4+g)*128+hh] = W_hh[g*512+j*128+hh, k*128+kk]
    Whh = W_hh.reshape(4, J, HH, KC, 128)              # [g, j, hh, k, kk]
    Whh_dev = np.ascontiguousarray(
        Whh.transpose(4, 3, 1, 0, 2).reshape(128, KC * J * 4 * HH)).astype(f)

    def chunk2(Wmat):  # [512, 512] -> [128, (k*4+m)*128+mm]
        Wr = Wmat.reshape(KC, 128, 4, 128)             # [k, kk, m, mm]
        return np.ascontiguousarray(Wr.transpose(1, 0, 2, 3).reshape(128, 2048))

    hb1_dev = chunk2(inp["hb1_W"].astype(np.float64)).astype(f)
    hz1_dev = chunk2(inp["hz1_W"].astype(np.float64)).astype(f)
    hb1b_dev = np.ascontiguousarray(
        inp["hb1_b"].astype(np.float64).reshape(4, 128).T).astype(f)   # [mm, m]
    hz1b_dev = np.ascontiguousarray(
        inp["hz1_b"].astype(np.float64).reshape(4, 128).T).astype(f)
    hb2_dev = np.ascontiguousarray(
        inp["hb2_W"].astype(np.float64).reshape(KC, 128).T).astype(f)  # [kk, k]
    hb2_b = float(inp["hb2_b"][0])
    hz2 = inp["hz2_W"].astype(np.float64).reshape(KC, 128, L)          # [k, kk, l]
    hz2_dev = np.ascontiguousarray(hz2.transpose(1, 0, 2).reshape(128, KC * L)).astype(f)
    hz2b_row = inp["hz2_b"].astype(f).reshape(1, L)

    dec1 = inp["dec1_W"].astype(np.float64)            # [L, 256, 512]
    dec1b = inp["dec1_b"].astype(np.float64)           # [L, 512]
    W1e = np.einsum("dh,lhk->ldk", embed_W, dec1)      # [L, 128, 512]
    b1e = dec1b + np.einsum("h,lhk->lk", embed_b, dec1)
    # W1e_dev[dd, (l*4+m)*128+mm]
    W1e_dev = np.ascontiguousarray(
        W1e.reshape(L, 128, 4, 128).transpose(1, 0, 2, 3).reshape(128, L * 512)).astype(f)
    b1e_dev = np.ascontiguousarray(
        b1e.reshape(L, 4, 128).transpose(2, 0, 1).reshape(128, L * 4)).astype(f)

    dec2 = inp["dec2_W"].astype(np.float64)            # [L, 512, 512]
    # W2_dev[kk, ((l*4+k)*4+m)*128+mm]
    W2_dev = np.ascontiguousarray(
        dec2.reshape(L, KC, 128, 4, 128).transpose(2, 0, 1, 3, 4)
        .reshape(128, L * 2048)).astype(f)
    b2_dev = np.ascontiguousarray(
        inp["dec2_b"].astype(np.float64).reshape(L, 4, 128)
        .transpose(2, 0, 1).reshape(128, L * 4)).astype(f)

    dec3 = inp["dec3_W"].astype(np.float64)            # [L, 512, 32]
    W3_dev = np.ascontiguousarray(
        dec3.reshape(L, KC, 128, A).transpose(2, 0, 1, 3).reshape(128, L * KC * A)).astype(f)
    b3_dev = np.ascontiguousarray(
        inp["dec3_b"].astype(np.float64).T).astype(f)  # [a, l]

    I128 = np.eye(128, dtype=f)
    ones1 = np.ones((1, 4), dtype=f)
    # Esel[p, 4j+b] = 1 iff p == 32j+b : compacts Hb.T columns {32j+b}
    Esel = np.zeros((128, 16), dtype=f)
    for j in range(J):
        for b in range(B):
            Esel[32 * j + b, 4 * j + b] = 1.0

    return dict(M1=M1_dev, M2p=M2p_dev, Whh=Whh_dev, hb1=hb1_dev, hb1b=hb1b_dev,
                hb2=hb2_dev, hz1=hz1_dev, hz1b=hz1b_dev, hz2=hz2_dev,
                hz2b=hz2b_row, W1e=W1e_dev, b1e=b1e_dev, W2=W2_dev, b2=b2_dev,
                W3=W3_dev, b3=b3_dev, I128=I128, ones1=ones1, Esel=Esel), hb2_b


def _pack_core_inputs(inp, gb, gz, core):
    """Per-core activation tensors. tok col = t_hi*128 + tt*4 + b."""
    f = np.float32
    b0 = core * B
    x = np.asarray(inp["inputs"][b0:b0 + B], dtype=f)          # [4, 128, 128]
    act = np.asarray(inp["actions"][b0:b0 + B]).astype(np.int64)
    lens = np.asarray(inp["lengths"][b0:b0 + B]).astype(np.int64)

    # xT[d, t_hi*128 + tt*4 + b] = x[b, t_hi*32+tt, d]
    xr = x.reshape(B, 4, 32, D)                                # [b, t_hi, tt, d]
    xT = np.ascontiguousarray(xr.transpose(3, 1, 2, 0).reshape(D, TOK)).astype(f)
    aoh_full = np.zeros((A, B, T), dtype=f)
    for b in range(B):
        aoh_full[act[b], b, np.arange(T)] = 1.0
    aohr = aoh_full.reshape(A, B, 4, 32)                       # [a, b, t_hi, tt]
    aoh = np.ascontiguousarray(aohr.transpose(0, 2, 3, 1).reshape(A, TOK)).astype(f)

    gb_c = gb[:, b0:b0 + B, :]                                 # [3, 4, 128]
    gb_dev = np.ascontiguousarray(gb_c.reshape(1, 3 * B * T)).astype(f)
    gz_c = gz[:, b0:b0 + B, :]                                 # [4, 4, 16]
    gz_dev = np.ascontiguousarray(gz_c.transpose(1, 0, 2).reshape(B, SEGS * L)).astype(f)

    loh = np.zeros((1, B * T), dtype=f)
    for b in range(B):
        loh[0, b * T + (int(lens[b]) - 1)] = 1.0
    return dict(xT=xT, aoh=aoh, gb=gb_dev, gz=gz_dev, loh=loh)


# ----------------------------------------------------------------------------
# Device program
# ----------------------------------------------------------------------------

def _build_program(hb2_b, skip_decoder=False, skip_boundary=False, nsegs=SEGS):
    import concourse.bass as bass
    import concourse.mybir as mybir
    from concourse import bacc, tile

    f32 = mybir.dt.float32
    AF = mybir.ActivationFunctionType
    OP = mybir.AluOpType
    AX = mybir.AxisListType

    nc = bacc.Bacc(None, target_bir_lowering=False, debug=False)

    # ---- DRAM I/O ----
    di = {}
    def d_in(name, shape):
        di[name] = nc.dram_tensor(name, list(shape), f32, kind="ExternalInput")
        return di[name]

    for name, shape in [
        ("xT", (D, TOK)), ("aoh", (A, TOK)), ("M1", (D, G)), ("M2p", (A, G)),
        ("Whh", (128, KC * J * 4 * HH)), ("hb1", (128, 2048)), ("hb1b", (128, 4)),
        ("hb2", (128, 4)), ("hz1", (128, 2048)), ("hz1b", (128, 4)),
        ("hz2", (128, KC * L)), ("hz2b", (1, L)), ("W1e", (128, L * 512)),
        ("b1e", (128, L * 4)), ("W2", (128, L * 2048)), ("b2", (128, L * 4)),
        ("W3", (128, L * KC * A)), ("b3", (A, L)), ("I128", (128, 128)),
        ("Esel", (128, 16)),
        ("ones1", (1, 4)), ("gb", (1, 3 * B * T)), ("gz", (B, SEGS * L)),
        ("loh", (1, B * T)),
    ]:
        d_in(name, shape)

    outs_dr = nc.dram_tensor("outs_dr", [L, B, T, A], f32)
    o_enc = nc.dram_tensor("o_enc", [SEGS, B, T, H], f32, kind="ExternalOutput")
    o_rec = nc.dram_tensor("o_rec", [SEGS, B, T, A], f32, kind="ExternalOutput")
    o_mask = nc.dram_tensor("o_mask", [SEGS - 1, B, T], f32, kind="ExternalOutput")
    o_lb = nc.dram_tensor("o_lb", [SEGS - 1, B, T], f32, kind="ExternalOutput")
    o_sb = nc.dram_tensor("o_sb", [SEGS, B, T], f32, kind="ExternalOutput")
    o_lz = nc.dram_tensor("o_lz", [SEGS, B, L], f32, kind="ExternalOutput")
    o_z = nc.dram_tensor("o_z", [SEGS, B, L], f32, kind="ExternalOutput")

    with tile.TileContext(nc) as tc:
        with (
            tc.tile_pool(name="w", bufs=1) as wp,
            tc.tile_pool(name="stream", bufs=2) as sp,
            tc.tile_pool(name="pg", bufs=1, space="PSUM") as pgp,
            tc.tile_pool(name="pt", bufs=1, space="PSUM") as ptp,
            tc.tile_pool(name="pd", bufs=2, space="PSUM") as pdp,
            tc.tile_pool(name="ps", bufs=2, space="PSUM") as psp,
        ):
            # ---- resident SBUF tiles + input DMAs ----
            sb = {}
            for name in ["xT", "aoh", "M1", "M2p", "Whh", "hb1", "hb1b", "hb2",
                         "hz1", "hz1b", "hz2", "hz2b", "b1e", "b2", "W3", "b3",
                         "I128", "Esel", "ones1", "gb", "gz", "loh"]:
                t = wp.tile(list(di[name].shape), f32, tag=name)
                nc.sync.dma_start(t[:], di[name][:])
                sb[name] = t

            Gin = wp.tile([128, KC * J * 4 * HH], f32, tag="Gin")   # [ (tt,b), (t_hi,j,g,hh) ]
            hTs = wp.tile([128, T * 16], f32, tag="hTs")            # [hh, (t,j,b)]
            S_if = wp.tile([128, 256], f32, tag="S_if")
            S_o = wp.tile([128, 128], f32, tag="S_o")
            TC = wp.tile([128, 256], f32, tag="TC")                 # [tg | c]
            P2 = wp.tile([128, 256], f32, tag="P2")
            Cn = wp.tile([128, 128], f32, tag="Cn")
            Tc = wp.tile([128, 128], f32, tag="Tc")
            OM = wp.tile([128, 128], f32, tag="OM")
            Hb = wp.tile([128, 128], f32, tag="Hb")
            mask_sb = wp.tile([128, T], f32, tag="mask_sb")
            ones128 = wp.tile([1, 128], f32, tag="ones128")
            roT = wp.tile([128, 16], f32, tag="roT")
            hzT = wp.tile([128, 16], f32, tag="hzT")
            lb_sb = wp.tile([1, TOK], f32, tag="lb_sb")
            sbn = wp.tile([1, TOK], f32, tag="sbn")
            sbe = wp.tile([1, TOK], f32, tag="sbe")
            sb_row = wp.tile([1, TOK], f32, tag="sb_row")
            cum = wp.tile([1, TOK], f32, tag="cum")
            lncum = wp.tile([1, TOK], f32, tag="lncum")
            logacc = wp.tile([1, TOK], f32, tag="logacc")
            mask_row = wp.tile([1, TOK], f32, tag="mask_row")
            zero_row = wp.tile([1, TOK], f32, tag="zero_row")
            mx = wp.tile([1, B], f32, tag="mx")
            ssum = wp.tile([1, B], f32, tag="ssum")
            rs = wp.tile([1, B], f32, tag="rs")
            lz_sb = wp.tile([B, L], f32, tag="lz_sb")
            zn = wp.tile([B, L], f32, tag="zn")
            ze = wp.tile([B, L], f32, tag="ze")
            z_sb = wp.tile([B, L], f32, tag="z_sb")
            nm = wp.tile([B, 1], f32, tag="nm")
            zs = wp.tile([B, 1], f32, tag="zs")
            rz = wp.tile([B, 1], f32, tag="rz")
            rec_sb = wp.tile([1, 1024], f32, tag="rec_sb")
            zT_all = [wp.tile([128, B], f32, tag=f"zT{s}", name=f"zT{s}")
                      for s in range(SEGS)]

            nc.vector.memset(ones128[:], 1.0)
            eps_t = wp.tile([128, 1], f32, tag="eps_t")
            hb2b_t = wp.tile([128, 1], f32, tag="hb2b_t")
            nc.vector.memset(eps_t[:], EPS)
            nc.vector.memset(hb2b_t[:], hb2_b)
            nc.vector.memset(mask_sb[:], 1.0)
            nc.vector.memset(logacc[:], 0.0)
            nc.vector.memset(zero_row[:], 0.0)
            nc.vector.memset(Hb[:], 0.0)

            # persistent psum tiles (ping-pong)
            PG = [pgp.tile([128, 512], f32, tag=f"pg{i}", name=f"pg{i}")
                  for i in range(2)]
            PT = [ptp.tile([128, 128], f32, tag=f"pt{i}", name=f"pt{i}")
                  for i in range(2)]
            nc.vector.memset(PG[0][:], 0.0)
            nc.vector.memset(PG[1][:], 0.0)

            # ---- Gin precompute ----
            for thi in range(4):
                for j in range(J):
                    pgi = pdp.tile([128, 512], f32, tag="dec_ps")
                    nc.tensor.matmul(pgi[:], sb["xT"][:, thi * 128:(thi + 1) * 128],
                                     sb["M1"][:, j * 512:(j + 1) * 512],
                                     start=True, stop=False)
                    nc.tensor.matmul(pgi[:], sb["aoh"][:, thi * 128:(thi + 1) * 128],
                                     sb["M2p"][:, j * 512:(j + 1) * 512],
                                     start=False, stop=True)
                    nc.scalar.copy(Gin[:, (thi * J + j) * 512:(thi * J + j + 1) * 512],
                                   pgi[:])

            # ---- helper: per-(b) softmax over T on a [1, TOK] row ----
            def softmax_rowT(dst, src):
                for b in range(B):
                    sl = slice(b * T, (b + 1) * T)
                    nc.vector.tensor_reduce(mx[0:1, b:b + 1], src[0:1, sl],
                                            AX.X, OP.max, negate=True)
                    nc.scalar.activation(sbe[0:1, sl], src[0:1, sl], AF.Exp,
                                         bias=mx[0:1, b:b + 1], scale=1.0,
                                         accum_out=ssum[0:1, b:b + 1])
                    nc.vector.reciprocal(rs[0:1, b:b + 1], ssum[0:1, b:b + 1])
                    nc.vector.tensor_scalar_mul(dst[0:1, sl], sbe[0:1, sl],
                                                rs[0:1, b:b + 1])

            # ================= segment loop =================
            for s in range(nsegs):
                # -- recurrence --
                nc.vector.memset(TC[:], 0.0)       # c := 0 (and tg scratch)
                for t in range(T):
                    tt, thi = t % 32, t // 32
                    pg = PG[t % 2]
                    for j in range(J):
                        nc.tensor.matmul(
                            pg[32 * j:32 * j + B, :],
                            sb["I128"][:, tt * 4:tt * 4 + 4],
                            Gin[:, (thi * J + j) * 512:(thi * J + j + 1) * 512],
                            start=True, stop=(t == 0), tile_position=(0, 32 * j))
                    if t > 0:
                        for k in range(KC):
                            for j in range(J):
                                nc.tensor.matmul(
                                    pg[32 * j:32 * j + B, :],
                                    hTs[:, (t - 1) * 16 + 4 * k:(t - 1) * 16 + 4 * k + 4],
                                    sb["Whh"][:, (k * J + j) * 512:(k * J + j + 1) * 512],
                                    start=False, stop=(k == KC - 1),
                                    tile_position=(0, 32 * j))
                    # cell math
                    nc.scalar.activation(S_if[:], pg[:, 0:256], AF.Sigmoid)
                    nc.scalar.activation(TC[:, 0:128], pg[:, 256:384], AF.Tanh)
                    nc.scalar.activation(S_o[:], pg[:, 384:512], AF.Sigmoid)
                    nc.vector.tensor_mul(P2[:], S_if[:], TC[:])
                    nc.vector.tensor_add(Cn[:], P2[:, 0:128], P2[:, 128:256])
                    nc.vector.tensor_scalar_mul(OM[:], S_o[:], mask_sb[:, t:t + 1])
                    nc.scalar.activation(Tc[:], Cn[:], AF.Tanh)
                    nc.vector.tensor_mul(Hb[:], OM[:], Tc[:])
                    nc.vector.tensor_scalar_mul(TC[:, 128:256], Cn[:],
                                                mask_sb[:, t:t + 1])
                    pt = PT[t % 2]
                    nc.tensor.matmul(pt[:, 0:16], Hb[:], sb["Esel"][:],
                                     start=True, stop=True)
                    nc.vector.tensor_copy(hTs[:, t * 16:(t + 1) * 16], pt[:, 0:16])

                # enc output DMA: o_enc[s][b,t,h] <- hTs[hh, (t,j,b)]
                for b in range(B):
                    nc.sync.dma_start(
                        o_enc[s, b].rearrange("t (j hh) -> hh t j", j=4),
                        hTs[:].rearrange("p (t j b) -> p t j b", j=4, b=B)[:, :, :, b])

                # -- boundary head --
                if skip_boundary:
                    continue
                if s < SEGS - 1:
                    encT = hTs[:].rearrange("p (t j b) -> p j b t", j=4, b=B)
                    plb = psp.tile([1, 512], f32, tag="small_ps")
                    for m in range(4):
                        ph = pdp.tile([128, 512], f32, tag="dec_ps")
                        for k in range(KC):
                            nc.tensor.matmul(
                                ph[:], sb["hb1"][:, (k * 4 + m) * 128:(k * 4 + m + 1) * 128],
                                encT[:, k], start=(k == 0), stop=(k == KC - 1))
                        hidc = sp.tile([128, 512], f32, tag="hidc")
                        nc.scalar.activation(hidc[:], ph[:],
                                             AF.Relu, bias=sb["hb1b"][:, m:m + 1])
                        nc.tensor.matmul(plb[:], sb["hb2"][:, m:m + 1], hidc[:],
                                         start=(m == 0), stop=(m == 3),
                                         skip_group_check=True)
                    nc.scalar.activation(lb_sb[:], plb[:], AF.Identity,
                                         bias=hb2b_t[0:1, 0:1])
                    for b in range(B):
                        nc.vector.memset(lb_sb[0:1, b * T:b * T + 1], NEG_INF)
                    nc.sync.dma_start(o_lb[s].rearrange("b t -> (b t)").unsqueeze(0),
                                      lb_sb[:])
                    nc.vector.tensor_add(sbn[:], lb_sb[:],
                                         sb["gb"][0:1, s * TOK:(s + 1) * TOK])
                    softmax_rowT(sb_row, sbn)
                    sbr = sb_row
                else:
                    sbr = sb["loh"]
                nc.sync.dma_start(o_sb[s].rearrange("b t -> (b t)").unsqueeze(0), sbr[:])

                # -- readout + latent head --
                sbB = psp.tile([128, 512], f32, tag="small_ps")
                nc.tensor.matmul(sbB[:], ones128[:], sbr[0:1, :],
                                 start=True, stop=True)
                for j in range(J):
                    pr = sp.tile([128, 4 * 127], f32, tag="pr")
                    nc.vector.tensor_mul(
                        pr[:].rearrange("p (b t) -> p b t", b=B),
                        hTs[:].rearrange("p (t j b) -> p j b t", j=4, b=B)
                        [:, j, :, 0:127],
                        sbB[:].rearrange("p (b t) -> p b t", b=B)[:, :, 1:128])
                    nc.vector.tensor_reduce(
                        roT[:, 4 * j:4 * j + 4],
                        pr[:].rearrange("p (b t) -> p b t", b=B),
                        AX.X, OP.add)
                for m in range(4):
                    phz = psp.tile([128, 4], f32, tag="small_ps")
                    for k in range(KC):
                        nc.tensor.matmul(
                            phz[:], sb["hz1"][:, (k * 4 + m) * 128:(k * 4 + m + 1) * 128],
                            roT[:, 4 * k:4 * k + 4], start=(k == 0), stop=(k == KC - 1))
                    nc.scalar.activation(hzT[:, m * 4:(m + 1) * 4], phz[:],
                                         AF.Relu, bias=sb["hz1b"][:, m:m + 1])
                plz = psp.tile([B, L], f32, tag="small_ps")
                for k in range(KC):
                    nc.tensor.matmul(plz[:], hzT[:, k * 4:k * 4 + 4],
                                     sb["hz2"][:, k * L:(k + 1) * L],
                                     start=(k == 0), stop=False)
                nc.tensor.matmul(plz[:], sb["ones1"][:], sb["hz2b"][:],
                                 start=False, stop=True)
                nc.scalar.copy(lz_sb[:], plz[:])
                nc.sync.dma_start(o_lz[s][:], lz_sb[:])
                nc.vector.tensor_add(zn[:], lz_sb[:], sb["gz"][:, s * L:(s + 1) * L])
                nc.vector.tensor_reduce(nm[:], zn[:], AX.X, OP.max, negate=True)
                nc.scalar.activation(ze[:], zn[:], AF.Exp, bias=nm[:, 0:1],
                                     scale=1.0, accum_out=zs[:, 0:1])
                nc.vector.reciprocal(rz[:], zs[:])
                nc.vector.tensor_scalar_mul(z_sb[:], ze[:], rz[:, 0:1])
                nc.sync.dma_start(o_z[s][:], z_sb[:])
                pzt = psp.tile([L, B], f32, tag="small_ps")
                nc.tensor.transpose(pzt[:], z_sb[:], sb["I128"][0:B, 0:B])
                nc.scalar.copy(zT_all[s][0:L, :], pzt[:])

                # -- mask update --
                if s < SEGS - 1:
                    for b in range(B):
                        sl = slice(b * T, (b + 1) * T)
                        nc.vector.tensor_tensor_scan(
                            cum[0:1, sl], sbr[0:1, sl], zero_row[0:1, sl],
                            0.0, OP.add, OP.add)
                    nc.scalar.activation(lncum[:], cum[:], AF.Ln,
                                         bias=eps_t[0:1, 0:1])
                    nc.vector.tensor_add(logacc[:], logacc[:], lncum[:])
                    nc.scalar.activation(mask_row[:], logacc[:], AF.Exp)
                    nc.sync.dma_start(o_mask[s].rearrange("b t -> (b t)").unsqueeze(0),
                                      mask_row[:])
                    for j in range(J):
                        for b in range(B):
                            nc.sync.dma_start(mask_sb[32 * j + b:32 * j + b + 1, :],
                                              mask_row[0:1, b * T:(b + 1) * T])

            # ================= decoder (segment-independent) =================
            for l in range(0 if skip_decoder else L):
                w1 = sp.tile([128, 512], f32, tag="w1l")
                nc.sync.dma_start(w1[:], di["W1e"][:, l * 512:(l + 1) * 512])
                w2h = []
                for c in range(2):
                    w2c = sp.tile([128, 1024], f32, tag="w2l", name=f"w2c{c}")
                    nc.sync.dma_start(
                        w2c[:], di["W2"][:, l * 2048 + c * 1024:l * 2048 + (c + 1) * 1024])
                    w2h.append(w2c)
                h1T = sp.tile([128, 4 * 512], f32, tag="h1T", bufs=1)
                h2T = sp.tile([128, 4 * 512], f32, tag="h2T", bufs=1)
                for m in range(4):
                    p1 = pdp.tile([128, 512], f32, tag="dec_ps")
                    nc.tensor.matmul(p1[:], w1[:, m * 128:(m + 1) * 128],
                                     sb["xT"][:], start=True, stop=True)
                    nc.scalar.activation(h1T[:, m * 512:(m + 1) * 512], p1[:],
                                         AF.Tanh, bias=sb["b1e"][:, l * 4 + m:l * 4 + m + 1])
                for m in range(4):
                    p2d = pdp.tile([128, 512], f32, tag="dec_ps")
                    for k in range(KC):
                        nc.tensor.matmul(
                            p2d[:],
                            w2h[k // 2][:, ((k % 2) * 4 + m) * 128:((k % 2) * 4 + m + 1) * 128],
                            h1T[:, k * 512:(k + 1) * 512],
                            start=(k == 0), stop=(k == KC - 1))
                    nc.scalar.activation(h2T[:, m * 512:(m + 1) * 512], p2d[:],
                                         AF.Tanh, bias=sb["b2"][:, l * 4 + m:l * 4 + m + 1])
                p3 = psp.tile([A, 512], f32, tag="small_ps")
                for k in range(KC):
                    nc.tensor.matmul(p3[:], sb["W3"][:, (l * 4 + k) * A:(l * 4 + k + 1) * A],
                                     h2T[:, k * 512:(k + 1) * 512],
                                     start=(k == 0), stop=(k == KC - 1))
                outl = sp.tile([A, 512], f32, tag="outl")
                nc.scalar.activation(outl[:], p3[:],
                                     AF.Identity, bias=sb["b3"][:, l:l + 1])
                for b in range(B):
                    nc.sync.dma_start(
                        outs_dr[l, b].rearrange("(thi tt) a -> a thi tt", thi=4),
                        outl[:].rearrange("a (thi tt b) -> a thi tt b",
                                          thi=4, b=B)[:, :, :, b])

            # ---- mixing: rec[s][b,t,a] = sum_l z[b,l] * outs[l,b,t,a] ----
            for thi in range(0 if skip_decoder else 4):
                for hb in range(2):          # pair of b values
                    OT2 = sp.tile([L, 2 * 32 * A], f32, tag="OT2", bufs=1)
                    for l in range(L):
                        # OT2[l, b'*1024 + tt*32 + a] <- outs_dr[l, 2hb+b', thi*32+tt, a]
                        nc.sync.dma_start(
                            OT2[l:l + 1, :],
                            outs_dr[l, 2 * hb:2 * hb + 2, thi * 32:(thi + 1) * 32, :]
                            .unsqueeze(0))
                    for s in range(SEGS):
                        for bl in range(2):
                            b = 2 * hb + bl
                            for c in range(2):
                                pm = psp.tile([1, 512], f32, tag="small_ps")
                                nc.tensor.matmul(
                                    pm[:], zT_all[s][0:L, b:b + 1],
                                    OT2[:, bl * 1024 + c * 512: bl * 1024 + (c + 1) * 512],
                                    start=True, stop=True)
                                nc.scalar.copy(rec_sb[0:1, c * 512:(c + 1) * 512], pm[:])
                            nc.sync.dma_start(
                                o_rec[s, b].rearrange("t a -> (t a)").unsqueeze(0)
                                [0:1, thi * 1024:(thi + 1) * 1024],
                                rec_sb[:])

    nc.compile()
    return nc


# ----------------------------------------------------------------------------
# Entry point
# ----------------------------------------------------------------------------

def benchmark(inputs, iters=10):
    """Time repeated device executions (inputs resident, no donation).

    Returns (best_ns, mean_ns). Requires all outputs fully written by the
    kernel (true here), since output buffers are not pre-zeroed per call.
    """
    import sys, time
    for p in ("/opt/trn_rl_repo", "/root/.axon_site/_ro/trn_rl_repo"):
        if os.path.isdir(p) and p not in sys.path:
            sys.path.insert(0, p)
    import jax
    import numpy as np
    from jax.sharding import Mesh, PartitionSpec
    from jax.experimental.shard_map import shard_map
    import concourse.mybir as mybir
    from concourse import bass2jax
    from concourse.bass2jax import _bass_exec_p, partition_id_tensor

    gb, gz = _gumbel_noise()
    shared, hb2_b = _pack_weights(inputs)
    if "nc" not in _prog_cache:
        _prog_cache["nc"] = _build_program(hb2_b)
    nc = _prog_cache["nc"]
    bass2jax.install_neuronx_cc_hook()

    in_maps = []
    for core in range(NC):
        m = dict(shared)
        m.update(_pack_core_inputs(inputs, gb, gz, core))
        in_maps.append(m)

    partition_name = nc.partition_id_tensor.name if nc.partition_id_tensor else None
    in_names, out_names, out_avals, zero_outs = [], [], [], []
    for alloc in nc.m.functions[0].allocations:
        if not isinstance(alloc, mybir.MemoryLocationSet):
            continue
        name = alloc.memorylocations[0].name
        if alloc.kind == "ExternalInput":
            if name != partition_name:
                in_names.append(name)
        elif alloc.kind == "ExternalOutput":
            shape = tuple(alloc.tensor_shape)
            dtype = mybir.dt.np(alloc.dtype)
            out_names.append(name)
            out_avals.append(jax.core.ShapedArray(shape, dtype))
            zero_outs.append(np.zeros(shape, dtype))
    n_params = len(in_names)
    in_names_all = in_names + out_names
    if partition_name is not None:
        in_names_all = in_names_all + [partition_name]

    def _body(*args):
        operands = list(args)
        if partition_name is not None:
            operands.append(partition_id_tensor())
        outs = _bass_exec_p.bind(
            *operands,
            out_avals=tuple(out_avals),
            in_names=tuple(in_names_all),
            out_names=tuple(out_names),
            lowering_input_output_aliases=(),
            sim_require_finite=True,
            sim_require_nnan=True,
            nc=nc,
        )
        return tuple(outs)

    devices = jax.devices()[:NC]
    mesh = Mesh(np.asarray(devices), ("core",))
    nin = n_params + len(out_names)
    sharded = jax.jit(
        shard_map(_body, mesh=mesh, in_specs=(PartitionSpec("core"),) * nin,
                  out_specs=(PartitionSpec("core"),) * len(out_names),
                  check_rep=False),
        keep_unused=True)
    concat_in = [np.concatenate([np.asarray(in_maps[c][nm]) for c in range(NC)], axis=0)
                 for nm in in_names]
    concat_in += [np.concatenate([z] * NC, axis=0) for z in zero_outs]
    dev_in = jax.device_put(concat_in)
    r = sharded(*dev_in)
    jax.block_until_ready(r)
    times = []
    for _ in range(iters):
        t0 = time.perf_counter()
        r = sharded(*dev_in)
        jax.block_until_ready(r)
        times.append(time.perf_counter() - t0)
    return int(min(times) * 1e9), int(np.mean(times) * 1e9)


def kernel(**inputs):
    import sys
    for p in ("/opt/trn_rl_repo", "/root/.axon_site/_ro/trn_rl_repo"):
        if os.path.isdir(p) and p not in sys.path:
            sys.path.insert(0, p)

    gb, gz = _gumbel_noise()
    shared, hb2_b = _pack_weights(inputs)

    if "nc" not in _prog_cache:
        _prog_cache["nc"] = _build_program(hb2_b)
    nc = _prog_cache["nc"]

    in_maps = []
    for core in range(NC):
        m = dict(shared)
        m.update(_pack_core_inputs(inputs, gb, gz, core))
        in_maps.append(m)

    from concourse.bass_utils import run_bass_kernel_spmd
    res = run_bass_kernel_spmd(nc, in_maps, list(range(NC)),
                               trace=bool(os.environ.get("KERNEL_TRACE")))
    _prog_cache["exec_time_ns"] = res.exec_time_ns
    _prog_cache["profile_json"] = res.profile_json
    outs = res.results

    encs = np.concatenate([r["o_enc"] for r in outs], axis=1)
    recs = np.concatenate([r["o_rec"] for r in outs], axis=1)
    masks = np.concatenate([r["o_mask"] for r in outs], axis=1)
    lbs = np.concatenate([r["o_lb"] for r in outs], axis=1)
    sbs = np.concatenate([r["o_sb"] for r in outs], axis=1)
    lzs = np.concatenate([r["o_lz"] for r in outs], axis=1)
    zs = np.concatenate([r["o_z"] for r in outs], axis=1)
    return (encs, recs, masks, lbs, sbs, lzs, zs)
